# revision 31
# baseline (speedup 1.0000x reference)
"""Trainium2 Bass kernel for nn_EnhancedTransformerBlock_51917564674691.

Reference block (B=4, S=2048, D=256):
  x_global = global_mha(x, 8 heads, hd=32)          # dense S x S attention
  x_local  = local_mha(x, 4 heads, hd=64, window=5) # banded attention
  x_fused  = MLP_512(silu) over concat([x_global, x_local])
  x        = LN(x + x_fused); x = LN(x + FFN(x)); return x

Sharding: 8 cores = 4 batches x 2 sequence-halves. Each core computes the
full-batch K/V for global attention (needs all 2048 keys) and produces the
output for its 1024 tokens.

Layouts: "T-layout" = [feature partitions, token free] throughout the matmul
chain; host pre-transposes x and all weights into SBUF-image form so every
DMA is a contiguous [128, F] block. Attention internals are bf16 (fp32 PSUM
accumulation everywhere).

Pipelining: the global-attention score PSUM is split into two [128,1024]
half-tiles (2 banks each, double-buffered) so the PE score matmuls of key
tile kt+1 overlap the softmax exp of key tile kt on the scalar engine; the
AV accumulation for the second half lags one iteration so the PE never
waits on an in-flight exp. Softmax denominators come from ones-columns in
the V weights; divisions use the fast DVE reciprocal approximation.
Elementwise work is spread across Vector, GpSimd(Pool) and Scalar engines.
"""
import os
import numpy as np
import ml_dtypes

import concourse.bass as bass
import concourse.tile as tile
from concourse import bacc, mybir
from concourse.bass_utils import run_bass_kernel_spmd
from concourse.masks import make_identity

P = 128
BF = mybir.dt.bfloat16
F32 = mybir.dt.float32
FP8 = mybir.dt.float8e4
I8 = mybir.dt.int8
BF_NP = ml_dtypes.bfloat16

B, S, D = 4, 2048, 256
TQ = 1024           # tokens per core
XQ = 1152           # padded x_q length (own tokens + halo, zero padded)
NQT = 2             # global q tiles of 512
NKT = 16            # global key tiles of 128
GSC = 1.0 / np.sqrt(32.0)   # global attention scale
LSC = 0.125                 # local attention scale (1/sqrt(64))
# Schraudolph fast-exp constants targeting a bf16 bit pattern:
# bf16bits(exp(GSC*x)) ~= round(x * 128*GSC*log2(e) + (127*128 - 5.5))
A16 = float(128.0 * GSC * np.log2(np.e))
B16 = 16250.5
LB = 124            # local block queries
NLB = 9             # local blocks (9*124 = 1116 >= 1024)
EPS = 1e-5

AF = mybir.ActivationFunctionType
TT = mybir.AluOpType

# name -> (shape, np dtype) of per-core DRAM inputs (all SBUF-image [128, F])
INPUT_SPECS = {
    "xkvT": ((P, 2 * 2048), BF_NP),   # x[b].T            (full batch, T-layout)
    "xqT": ((P, 2 * XQ), BF_NP),      # x_q.T padded      (own + halo, T-layout)
    "xownN": ((P, 8 * 256), np.float32),  # x own tokens + fus_b2 (N-layout)
    "wgq": ((P, 2 * 256), BF_NP),
    "wgk": ((P, 2 * 256), BF_NP),
    "wgv": ((P, 2 * 256), BF_NP),
    "wtqk": ((P, 2 * 512), BF_NP),
    "wtv": ((P, 2 * 256), BF_NP),
    "wgo": ((P, 2 * 256), BF_NP),
    "wto": ((P, 2 * 256), BF_NP),
    "wf1": ((P, 4 * 512), BF_NP),
    "wf2": ((P, 4 * 256), BF_NP),
    "wn1": ((P, 2 * 512), BF_NP),
    "wn2": ((P, 4 * 256), BF_NP),
    "bgq": ((P, 2), np.float32),
    "bgk": ((P, 2), np.float32),
    "btqk": ((P, 4), np.float32),
    "bgo": ((P, 2), np.float32),
    "bto": ((P, 2), np.float32),
    "bf1": ((P, 4), np.float32),
    "nbf1": ((P, 4), np.float32),
    "bn1": ((P, 4), np.float32),
    "nbn1": ((P, 4), np.float32),
    "bgv128": ((P, 256), np.float32),
    "btv128": ((P, 256), np.float32),
    "bn2128": ((P, 256), np.float32),
    "gng128": ((P, 256), np.float32),
    "gnb128": ((P, 256), np.float32),
    "fng128": ((P, 256), np.float32),
    "fnb128": ((P, 256), np.float32),
    "bandA": ((P, LB), BF_NP),
    "bandF": ((P, LB), BF_NP),
    "bandL": ((P, LB), BF_NP),
}


def _patch_act_tables():
    """Make Exp and Ln resolve to the combined natural_log_exp_and_others set
    so the table-load pass emits ONE load instead of thrashing between
    exp_and_others and natural_log (9 loads, ~2.7us each + ACT drains)."""
    import concourse.hw_specs as hs
    if getattr(hs, "_act_tables_patched", False):
        return
    orig = hs.get_activation_tables

    def patched(module_arch):
        t = dict(orig(module_arch))
        exp = mybir.ActivationFunctionType.Exp
        ln = mybir.ActivationFunctionType.Ln
        for name in ("exp_and_others", "exp_and_friends"):
            if name in t:
                t[name] = t[name] - {exp}
        if "natural_log" in t:
            t["natural_log"] = t["natural_log"] - {ln}
        return t

    hs.get_activation_tables = patched
    import concourse.bacc as bc
    bc.get_activation_tables = patched
    hs._act_tables_patched = True


def build(fast):
    _patch_act_tables()
    nc = bacc.Bacc("TRN2", target_bir_lowering=False, debug=False, num_devices=8)
    dram = {}
    for name, (shape, npdt) in INPUT_SPECS.items():
        dram[name] = nc.dram_tensor(
            name, list(shape), mybir.dt.from_np(np.dtype(npdt)), kind="ExternalInput"
        ).ap()
    out_dram = nc.dram_tensor("out", [P, 8 * 256], F32, kind="ExternalOutput").ap()

    with tile.TileContext(nc) as tc:
        _emit(nc, tc, dram, out_dram, fast)
    nc.compile()
    return nc


def _emit(nc, tc, dram, out_dram, fast):
    from contextlib import ExitStack
    ctx = ExitStack()

    cpool = ctx.enter_context(tc.tile_pool(name="const", bufs=1))
    wpool = ctx.enter_context(tc.tile_pool(name="work", bufs=1))
    spool = ctx.enter_context(tc.tile_pool(name="scratch", bufs=4))
    epool = ctx.enter_context(tc.tile_pool(name="exps", bufs=2))
    pp = ctx.enter_context(tc.tile_pool(name="ps", bufs=1, space="PSUM"))

    def _kernel_body():
            # ---- load constants / inputs --------------------------------------
            cin = {}
            for name, (shape, npdt) in INPUT_SPECS.items():
                t = cpool.tile(list(shape), mybir.dt.from_np(np.dtype(npdt)), tag=name)
                nc.sync.dma_start(t[:], dram[name])
                cin[name] = t

            ones_bf = cpool.tile([P, 64], BF, tag="ones_bf")
            nc.vector.memset(ones_bf[:], 1.0)
            ident = cpool.tile([P, P], F32, tag="ident")
            make_identity(nc, ident[:])

            # reshaped views of inputs
            xkvT = cin["xkvT"][:].rearrange("p (k n) -> p k n", k=2)     # [128,2,2048] bf
            xqT = cin["xqT"][:].rearrange("p (k n) -> p k n", k=2)       # [128,2,1152]
            xownN = cin["xownN"][:].rearrange("p (t f) -> p t f", t=8)   # [128,8,256] f32
            w = {k: cin[k][:].rearrange("p (k2 n) -> p k2 n", k2=2)
                 for k in ("wgq", "wgk", "wgv", "wtqk", "wtv", "wgo", "wto", "wn1")}
            w["wf1"] = cin["wf1"][:].rearrange("p (k2 n) -> p k2 n", k2=4)
            w["wf2"] = cin["wf2"][:].rearrange("p (k2 n) -> p k2 n", k2=4)
            w["wn2"] = cin["wn2"][:].rearrange("p (k2 n) -> p k2 n", k2=4)

            # ---- persistent intermediates ------------------------------------
            qT = wpool.tile([P, 2, 1024], BF, tag="qT")
            kT = wpool.tile([P, 2, 2048], BF, tag="kT")
            v_aug = wpool.tile([P, NKT, 8, 64], BF, tag="v_aug")
            qkL = wpool.tile([P, 4, XQ], BF, tag="qkL")
            vL = wpool.tile([P, NLB, 256], BF, tag="vL")
            g_oT = wpool.tile([P, 2, 1024], BF, tag="g_oT")
            l_oT = wpool.tile([P, 2, 1024], BF, tag="l_oT")
            combT = wpool.tile([P, 4, 1024], BF, tag="combT")
            h1s = wpool.tile([P, 4, 1024], BF, tag="h1s")
            x1N = wpool.tile([P, 8, 256], F32, tag="x1N")
            x1T = wpool.tile([P, 2, 1024], BF, tag="x1T")
            h2s = wpool.tile([P, 4, 1024], BF, tag="h2s")
            out_sb = wpool.tile([P, 8, 256], F32, tag="out_sb")

            def ps_sc():
                return pp.tile([P, 1024], F32, tag="sc", bufs=2, name="ps_sc")

            def ps_av():
                return pp.tile([P, 512], F32, tag="av", bufs=2, name="ps_av")

            def ps_sm():
                return pp.tile([P, 512], F32, tag="sm", bufs=2, name="ps_sm")

            def bias_bc(name, m, n):
                return cin[name][:, m:m + 1].to_broadcast([P, n])

            # ---- qkv projections (global) ------------------------------------
            # qT over own tokens first (x_q rows 2..1026)
            for m in range(2):
                for nt in range(2):
                    pm = ps_sm()
                    for k in range(2):
                        nc.tensor.matmul(pm[:], w["wgq"][:, k, 128 * m:128 * m + 128],
                                         xqT[:, k, 2 + 512 * nt:2 + 512 * nt + 512],
                                         start=(k == 0), stop=(k == 1))
                    if fast:
                        nc.vector.tensor_copy(qT[:, m, 512 * nt:512 * nt + 512], pm[:])
                    else:
                        nc.scalar.activation(qT[:, m, 512 * nt:512 * nt + 512], pm[:],
                                             AF.Identity, bias=cin["bgq"][:, m:m + 1])
            # kT = Wk @ x^T over full batch
            for m in range(2):
                for nt in range(4):
                    pm = ps_sm()
                    for k in range(2):
                        nc.tensor.matmul(pm[:], w["wgk"][:, k, 128 * m:128 * m + 128],
                                         xkvT[:, k, 512 * nt:512 * nt + 512],
                                         start=(k == 0), stop=(k == 1))
                    if fast:
                        nc.vector.tensor_copy(kT[:, m, 512 * nt:512 * nt + 512], pm[:])
                    else:
                        nc.scalar.activation(kT[:, m, 512 * nt:512 * nt + 512], pm[:],
                                             AF.Identity, bias=cin["bgk"][:, m:m + 1])
            # ---- qkv projections (local) -------------------------------------
            for m in range(4):
                for nt in range(3):
                    pm = ps_sm()
                    for k in range(2):
                        nc.tensor.matmul(pm[:, 0:384], w["wtqk"][:, k, 128 * m:128 * m + 128],
                                         xqT[:, k, 384 * nt:384 * nt + 384],
                                         start=(k == 0), stop=(k == 1))
                    nc.scalar.activation(qkL[:, m, 384 * nt:384 * nt + 384], pm[:, 0:384],
                                         AF.Identity, bias=cin["btqk"][:, m:m + 1])
            # v (N-layout, augmented with ones columns): v[key, f] over full batch
            nc.vector.memset(v_aug[:, :, :, 32:64], 1.0)
            for mt in range(16):
                pm = ps_sm()
                for k in range(2):
                    nc.tensor.matmul(pm[:, 0:256], xkvT[:, k, 128 * mt:128 * mt + 128],
                                     w["wgv"][:, k, :], start=(k == 0), stop=(k == 1))
                dst = v_aug[:, mt, :, 0:32]
                src = pm[:, 0:256].rearrange("p (h d) -> p h d", h=8)
                if fast:
                    nc.vector.tensor_copy(dst, src)
                else:
                    nc.vector.tensor_tensor(
                        dst, src,
                        cin["bgv128"][:].rearrange("p (h d) -> p h d", h=8), TT.add)
            for blk in range(NLB):
                pm = ps_sm()
                for k in range(2):
                    nc.tensor.matmul(pm[:, 0:256], xqT[:, k, 124 * blk:124 * blk + 128],
                                     w["wtv"][:, k, :], start=(k == 0), stop=(k == 1))
                if fast:
                    nc.vector.tensor_copy(vL[:, blk, :], pm[:, 0:256])
                else:
                    nc.vector.tensor_tensor(vL[:, blk, :], pm[:, 0:256],
                                            cin["btv128"][:], TT.add)

            # ---- local attention ---------------------------------------------
            # (pipelined: score PSUM halves double-buffered, exp per half,
            #  band mask on Pool, fast reciprocal on DVE)
            for blk in range(NLB):
                k0 = 124 * blk
                q0 = 2 + 124 * blk
                qn = 32 if blk == NLB - 1 else LB  # valid queries in this block
                eloc = epool.tile([P, 4, LB], BF, tag="eloc", bufs=2)
                for half in range(2):
                    psc = ps_sc()
                    pv = psc[:].rearrange("p (k n) -> p k n", k=2)
                    for r in range(2):
                        l = 2 * half + r
                        nc.tensor.matmul(pv[:, r, 0:LB],
                                         qkL[64 * r:64 * r + 64, 2 + half, k0:k0 + 128],
                                         qkL[64 * r:64 * r + 64, half, q0:q0 + LB],
                                         start=True, stop=True, tile_position=(64 * r, 0))
                    nc.scalar.activation(eloc[:, 2 * half:2 * half + 2, :],
                                         pv[:, :, 0:LB], AF.Exp, scale=LSC)
                band = ("bandF" if blk == 0 else
                        ("bandL" if blk == NLB - 1 else "bandA"))
                nc.vector.tensor_tensor(eloc[:], eloc[:],
                                        cin[band][:, None, :].to_broadcast([P, 4, LB]),
                                        TT.mult)
                pav = [ps_av(), ps_av()]
                pde = [ps_sm(), ps_sm()]
                for l in range(4):
                    pr, c = l // 2, l % 2
                    nc.tensor.matmul(pav[pr][64 * c:64 * c + 64, 0:LB],
                                     vL[:, blk, 64 * l:64 * l + 64], eloc[:, l, :],
                                     start=True, stop=True, tile_position=(0, 64 * c))
                    nc.tensor.matmul(pde[pr][64 * c:64 * c + 64, 0:LB],
                                     ones_bf[:], eloc[:, l, :],
                                     start=True, stop=True, tile_position=(0, 64 * c))
                for pr in range(2):
                    rec = spool.tile([P, LB], F32, tag="lrec", bufs=2)
                    nc.vector.reciprocal_approx_fast(rec[:], pde[pr][:, 0:LB])
                    nc.vector.tensor_tensor(l_oT[:, pr, k0:k0 + qn], pav[pr][:, 0:qn],
                                            rec[:, 0:qn], TT.mult)

            # ---- per-chunk: global attention + MLP tail ----------------------
            for qt in range(NQT):
                qsl = slice(512 * qt, 512 * qt + 512)
                for hg in range(2):
                    pav = [ps_av(), ps_av()]
                    lag = None  # (eg tile, kt) for the lagged half-B AV matmuls

                    def av_half(p2, eg, kt):
                        for c in range(2):
                            h = 4 * hg + 2 * p2 + c
                            # skip_group_check: CoreSim's zero-region tracker is
                            # partition-blind (any two concurrent groups per bank
                            # conflict); HW has per-element has_written bits and
                            # the 64-offset dual-group pattern is exact on HW.
                            nc.tensor.matmul(pav[p2][64 * c:64 * c + 64, :],
                                             v_aug[:, kt, h, :],
                                             eg[:, 512 * c:512 * c + 512],
                                             start=(kt == 0), stop=(kt == NKT - 1),
                                             tile_position=(0, 64 * c),
                                             skip_group_check=True)

                    for kt in range(NKT):
                        pscA = ps_sc()
                        for r in range(2):
                            nc.tensor.matmul(
                                pscA[:, 512 * r:512 * r + 512],
                                kT[32 * r:32 * r + 32, hg, 128 * kt:128 * kt + 128],
                                qT[32 * r:32 * r + 32, hg, qsl],
                                start=True, stop=True, tile_position=(32 * r, 0))
                        egA = epool.tile([P, 1024], BF, tag="egA", bufs=2)
                        nc.scalar.activation(egA[:], pscA[:], AF.Exp, scale=GSC)
                        pscB = ps_sc()
                        for r in range(2):
                            hc = 2 + r
                            nc.tensor.matmul(
                                pscB[:, 512 * r:512 * r + 512],
                                kT[32 * hc:32 * hc + 32, hg, 128 * kt:128 * kt + 128],
                                qT[32 * hc:32 * hc + 32, hg, qsl],
                                start=True, stop=True, tile_position=(32 * hc, 0))
                        # Schraudolph fast exp on DVE: scale+shift the score so
                        # the rounded int16 IS the bf16 bit pattern of
                        # exp(GSC*score) (softmax ratio cancels the
                        # piecewise-linear mantissa error)
                        egB = epool.tile([P, 1024], mybir.dt.int16, tag="egB",
                                         bufs=2)
                        nc.vector.tensor_scalar(egB[:], pscB[:], A16, B16,
                                                TT.mult, TT.add)
                        # AV lags one kt behind so the PE never waits on an
                        # in-flight exp
                        if lag is not None:
                            av_half(0, lag[0][:], lag[2])
                            av_half(1, lag[1][:].bitcast(BF), lag[2])
                        lag = (egA, egB, kt)
                    av_half(0, lag[0][:], lag[2])
                    av_half(1, lag[1][:].bitcast(BF), lag[2])

                    # normalize: one fast reciprocal per accumulator (denominator
                    # rows 32:64 / 96:128; extra rows are unused garbage)
                    for p2 in range(2):
                        rec = spool.tile([P, 512], F32, tag="grec", bufs=2)
                        nc.vector.reciprocal_approx_fast(rec[:], pav[p2][:])
                        nc.vector.tensor_tensor(g_oT[64 * p2:64 * p2 + 32, hg, qsl],
                                                pav[p2][0:32, :], rec[32:64, :], TT.mult)
                        nc.vector.tensor_tensor(g_oT[64 * p2 + 32:64 * p2 + 64, hg, qsl],
                                                pav[p2][64:96, :], rec[96:128, :], TT.mult)

                # ---- MLP tail in two 256-query column pipelines --------------
                # (halves the serial latency of the out-proj -> gemm1 -> silu
                #  -> gemm2 -> LN -> transpose -> FFN chain; the second half
                #  trails the first by one stage)
                for uu in range(2):
                    qsu = slice(512 * qt + 256 * uu, 512 * qt + 256 * uu + 256)
                    for m in range(2):
                        pm = ps_sm()
                        for k in range(2):
                            nc.tensor.matmul(pm[:, 0:256],
                                             w["wgo"][:, k, 128 * m:128 * m + 128],
                                             g_oT[:, k, qsu],
                                             start=(k == 0), stop=(k == 1))
                        if fast:
                            nc.scalar.activation(combT[:, m, qsu], pm[:, 0:256],
                                                 AF.Identity)
                        else:
                            nc.vector.tensor_tensor(combT[:, m, qsu], pm[:, 0:256],
                                                    bias_bc("bgo", m, 256), TT.add)
                    for m in range(2):
                        pm = ps_sm()
                        for k in range(2):
                            nc.tensor.matmul(pm[:, 0:256],
                                             w["wto"][:, k, 128 * m:128 * m + 128],
                                             l_oT[:, k, qsu],
                                             start=(k == 0), stop=(k == 1))
                        if fast:
                            nc.scalar.activation(combT[:, 2 + m, qsu], pm[:, 0:256],
                                                 AF.Identity)
                        else:
                            nc.vector.tensor_tensor(combT[:, 2 + m, qsu], pm[:, 0:256],
                                                    bias_bc("bto", m, 256), TT.add)

                    # fused MLP gemm1 + silu
                    for m in range(4):
                        pm = ps_sm()
                        for k in range(4):
                            nc.tensor.matmul(pm[:, 0:256],
                                             w["wf1"][:, k, 128 * m:128 * m + 128],
                                             combT[:, k, qsu],
                                             start=(k == 0), stop=(k == 3))
                        _silu(nc, spool, h1s[:, m, qsu], pm,
                              cin["bf1"], cin["nbf1"], m, fast)

                    # fused MLP gemm2 (N-layout out) + residual + LN1
                    _g2_res_ln(nc, spool, pp, cin, qt, uu, h1s, w["wf2"], None,
                               xownN, "gn", x1N, x1N_src=None, fast=fast)

                    # transpose x1N pair -> x1T
                    for tt in (2 * uu, 2 * uu + 1):
                        ta = 4 * qt + tt
                        for fh in range(2):
                            ptr = ps_sm()
                            nc.tensor.transpose(ptr[:, 0:128],
                                                x1N[:, ta, 128 * fh:128 * fh + 128],
                                                ident[:])
                            nc.scalar.activation(x1T[:, fh, 128 * ta:128 * ta + 128],
                                                 ptr[:, 0:128], AF.Identity)

                    # FFN gemm1 + silu
                    for m in range(4):
                        pm = ps_sm()
                        for k in range(2):
                            nc.tensor.matmul(pm[:, 0:256],
                                             w["wn1"][:, k, 128 * m:128 * m + 128],
                                             x1T[:, k, qsu],
                                             start=(k == 0), stop=(k == 1))
                        _silu(nc, spool, h2s[:, m, qsu], pm,
                              cin["bn1"], cin["nbn1"], m, fast)

                    # FFN gemm2 + residual(x1N) + LN2 -> out_sb
                    _g2_res_ln(nc, spool, pp, cin, qt, uu, h2s, w["wn2"], "bn2128",
                               None, "fn", out_sb, x1N_src=x1N, fast=fast)

                    # store half-chunk
                    nc.sync.dma_start(
                        out_dram[:, 1024 * qt + 512 * uu:1024 * qt + 512 * uu + 512],
                        out_sb[:, 4 * qt + 2 * uu:4 * qt + 2 * uu + 2, :]
                        .rearrange("p t f -> p (t f)"))

    REPEAT = int(os.environ.get("KREPEAT", "1"))
    if REPEAT > 1:
        with tc.For_i(0, REPEAT, 1):
            _kernel_body()
    else:
        _kernel_body()
    ctx.close()


def _silu(nc, spool, out_ap, pm, b_t, nb_t, m, fast):
    """out = silu(pm + b) where b is per-partition bias column m.

    silu(y) = y / (1 + exp(-y)); exp on ACT, then +1, fast reciprocal and
    the final (pm + b) * r on DVE."""
    src = pm[:, 0:256]
    e = spool.tile([P, 256], F32, tag="se", name="se", bufs=2)
    bias = 0.0 if fast else nb_t[:, m:m + 1]
    nc.scalar.activation(e[:], src, AF.Exp, bias=bias, scale=-1.0)
    nc.vector.tensor_scalar_add(e[:], e[:], 1.0)
    r = spool.tile([P, 256], F32, tag="sr", name="sr", bufs=2)
    nc.vector.reciprocal_approx_fast(r[:], e[:])
    if fast:
        nc.vector.tensor_tensor(out_ap, src, r[:], TT.mult)
    else:
        nc.vector.scalar_tensor_tensor(out_ap, src, b_t[:, m:m + 1], r[:],
                                       TT.add, TT.mult)


def _g2_res_ln(nc, spool, pp, cin, qt, uu, hsrc, w2, b128_name, xownN, ln_prefix,
               dest, x1N_src, fast):
    """gemm2 (contract 512 -> 256, N-layout out) + bias + residual + layernorm
    for the token pair (2*uu, 2*uu+1) of chunk qt.

    residual = xownN[:, ta, :] (already includes fus_b2, host-folded) or
    x1N_src[:, ta, :] (+ b128 on device unless fast).
    dest[:, ta, :] = LN(res + gemm2_out [+ b128]) [* g128 + b128_ln]
    The (x - mu) * istd normalize runs as ONE scalar-engine activation with
    per-partition scale/bias; sums for mu/var come fused out of the two DVE
    scalar_tensor_tensor ops (accum_out)."""
    xrs = []
    mu_raw = spool.tile([P, 2], F32, tag="mu_raw", bufs=2)
    s2_raw = spool.tile([P, 2], F32, tag="s2_raw", bufs=2)
    for tt in range(2):
        ta = 4 * qt + 2 * uu + tt
        pm = pp.tile([P, 512], F32, tag="sm", bufs=2, name="ps_sm")
        for k in range(4):
            nc.tensor.matmul(pm[:, 0:256], hsrc[:, k, 128 * ta:128 * ta + 128],
                             w2[:, k, :], start=(k == 0), stop=(k == 3))
        res = xownN[:, ta, :] if xownN is not None else x1N_src[:, ta, :]
        src = pm[:, 0:256]
        if not fast and b128_name is not None:
            tb = spool.tile([P, 256], F32, tag="tb", name="tb", bufs=2)
            nc.vector.tensor_tensor(tb[:], src, cin[b128_name][:], TT.add)
            src = tb[:]
        xr = spool.tile([P, 256], F32, tag=f"xr{tt}", name="xr")
        nc.vector.scalar_tensor_tensor(xr[:], src, 0.0, res, TT.add, TT.add,
                                       accum_out=mu_raw[:, tt:tt + 1])
        sq = spool.tile([P, 256], F32, tag="sq", name="sq", bufs=2)
        nc.vector.scalar_tensor_tensor(sq[:], xr[:], 0.0, xr[:], TT.add, TT.mult,
                                       accum_out=s2_raw[:, tt:tt + 1])
        xrs.append(xr)
    mu = spool.tile([P, 2], F32, tag="mu", bufs=2)
    var = spool.tile([P, 2], F32, tag="var", bufs=2)
    istd = spool.tile([P, 2], F32, tag="istd", bufs=2)
    nmui = spool.tile([P, 2], F32, tag="nmui", bufs=2)
    nc.vector.tensor_scalar_mul(mu[:], mu_raw[:], 1.0 / 256.0)
    nc.vector.tensor_scalar_mul(var[:], s2_raw[:], 1.0 / 256.0)
    nc.vector.tensor_tensor(istd[:], mu[:], mu[:], TT.mult)
    nc.vector.tensor_tensor(var[:], var[:], istd[:], TT.subtract)
    nc.vector.tensor_scalar_add(var[:], var[:], EPS)
    nc.scalar.activation(var[:], var[:], AF.Ln)
    nc.scalar.activation(istd[:], var[:], AF.Exp, scale=-0.5)
    nc.vector.tensor_tensor(nmui[:], mu[:], istd[:], TT.mult)
    nc.vector.tensor_scalar_mul(nmui[:], nmui[:], -1.0)
    for tt in range(2):
        ta = 4 * qt + 2 * uu + tt
        xr = xrs[tt]
        if fast:
            nc.scalar.activation(dest[:, ta, :], xr[:], AF.Identity,
                                 bias=nmui[:, tt:tt + 1], scale=istd[:, tt:tt + 1])
        else:
            nc.scalar.activation(xr[:], xr[:], AF.Identity,
                                 bias=nmui[:, tt:tt + 1], scale=istd[:, tt:tt + 1])
            nc.vector.tensor_tensor(xr[:], xr[:], cin[ln_prefix + "g128"][:], TT.mult)
            nc.vector.tensor_tensor(dest[:, ta, :], xr[:], cin[ln_prefix + "b128"][:],
                                    TT.add)


# ======================================================================
# Host side
# ======================================================================

_NC = {}


def _get_nc(fast):
    if fast not in _NC:
        _NC[fast] = build(fast)
    return _NC[fast]


def _img_T(mat):
    """[R, C] fp32 (R = k*128) -> SBUF image [128, k*C] for T-layout tiles."""
    R, C = mat.shape
    k = R // 128
    return np.ascontiguousarray(
        mat.reshape(k, 128, C).transpose(1, 0, 2).reshape(128, k * C))


def _img_N(mat):
    """[T, F] (T = t*128) -> SBUF image [128, t*F] for N-layout tiles."""
    T, F = mat.shape
    t = T // 128
    return np.ascontiguousarray(
        mat.reshape(t, 128, F).transpose(1, 0, 2).reshape(128, t * F))


def _bias_cols(b):
    """[k*128] -> [128, k] per-partition column layout."""
    return np.ascontiguousarray(b.reshape(-1, 128).T)


def _in_maps(x, g_in_w, g_in_b, g_out_w, g_out_b,
             t_in_w, t_in_b, t_out_w, t_out_b,
             fus_w1, fus_b1, fus_w2, fus_b2,
             ffn_w1, ffn_b1, ffn_w2, ffn_b2,
             gn_g, gn_b, fn_g, fn_b):
    x = np.asarray(x, np.float32)
    f32 = lambda a: np.asarray(a, np.float32)
    bf = lambda a: np.asarray(a, np.float32).astype(BF_NP)

    # fast path: every remaining device-side bias is zero and LN affine
    # params are identity (guaranteed by the problem's input fills; the
    # generic path handles anything else)
    fast = bool(
        np.all(f32(g_in_b)[512:768] == 0) and np.all(f32(t_in_b)[512:768] == 0)
        and np.all(f32(g_out_b) == 0) and np.all(f32(t_out_b) == 0)
        and np.all(f32(ffn_b2) == 0)
        and np.all(f32(gn_g) == 1) and np.all(f32(gn_b) == 0)
        and np.all(f32(fn_g) == 1) and np.all(f32(fn_b) == 0))

    # shared (same on all cores) tensors
    shared = {
        "wgq": bf(_img_T(f32(g_in_w)[0:256].T)),
        "wgk": bf(_img_T(f32(g_in_w)[256:512].T)),
        "wgv": bf(_img_T(f32(g_in_w)[512:768].T)),
        "wtqk": bf(_img_T(f32(t_in_w)[0:512].T)),
        "wtv": bf(_img_T(f32(t_in_w)[512:768].T)),
        "wgo": bf(_img_T(f32(g_out_w).T)),
        "wto": bf(_img_T(f32(t_out_w).T)),
        "wf1": bf(_img_T(f32(fus_w1).T)),
        "wf2": bf(_img_T(f32(fus_w2).T)),
        "wn1": bf(_img_T(f32(ffn_w1).T)),
        "wn2": bf(_img_T(f32(ffn_w2).T)),
        "bgq": _bias_cols(f32(g_in_b)[0:256]),
        "bgk": _bias_cols(f32(g_in_b)[256:512]),
        "btqk": _bias_cols(f32(t_in_b)[0:512]),
        "bgo": _bias_cols(f32(g_out_b)),
        "bto": _bias_cols(f32(t_out_b)),
        "bf1": _bias_cols(f32(fus_b1)),
        "nbf1": _bias_cols(-f32(fus_b1)),
        "bn1": _bias_cols(f32(ffn_b1)),
        "nbn1": _bias_cols(-f32(ffn_b1)),
        "bgv128": np.ascontiguousarray(
            np.broadcast_to(f32(g_in_b)[512:768], (P, 256))),
        "btv128": np.ascontiguousarray(
            np.broadcast_to(f32(t_in_b)[512:768], (P, 256))),
        "bn2128": np.ascontiguousarray(np.broadcast_to(f32(ffn_b2), (P, 256))),
        "gng128": np.ascontiguousarray(np.broadcast_to(f32(gn_g), (P, 256))),
        "gnb128": np.ascontiguousarray(np.broadcast_to(f32(gn_b), (P, 256))),
        "fng128": np.ascontiguousarray(np.broadcast_to(f32(fn_g), (P, 256))),
        "fnb128": np.ascontiguousarray(np.broadcast_to(f32(fn_b), (P, 256))),
    }
    # band mask: key row j valid for query qq iff qq <= j <= qq+4
    jj = np.arange(P)[:, None]
    qq = np.arange(LB)[None, :]
    bandA = ((qq <= jj) & (jj <= qq + 4)).astype(np.float32)
    bandF = bandA.copy()
    bandF[0:2] = 0.0           # keys at tokens -2, -1 (first block, first half)
    bandL = bandA.copy()
    bandL[34:36] = 0.0         # block-8 keys x_q rows 1026, 1027 (= S, S+1)
    shared["bandA"] = bandA.astype(BF_NP)

    in_maps = []
    for c in range(8):
        b, hh = c // 2, c % 2
        t0 = 1024 * hh
        xb = x[b]                                    # [2048, 256]
        xq = np.zeros((XQ + 4, D), np.float32)       # rows = x_q tokens t0-2 ..
        lo, hi = max(0, t0 - 2), min(S, t0 + XQ + 2)
        xq[lo - (t0 - 2):hi - (t0 - 2)] = xb[lo:hi]
        xq = xq[:XQ]                                 # guard: only XQ rows used
        m = dict(shared)
        m["xkvT"] = bf(_img_T(xb.T))
        m["xqT"] = bf(_img_T(xq.T))
        m["xownN"] = _img_N(xb[t0:t0 + 1024] + f32(fus_b2)[None, :])
        m["bandF"] = (bandF if hh == 0 else bandA).astype(BF_NP)
        m["bandL"] = (bandL if hh == 1 else bandA).astype(BF_NP)
        in_maps.append(m)
    return in_maps, fast


def _assemble(results):
    out = np.zeros((B, S, D), np.float32)
    for c in range(8):
        b, hh = c // 2, c % 2
        img = results[c]["out"]                      # [128, 2048]
        chunk = img.reshape(P, 8, 256).transpose(1, 0, 2).reshape(1024, 256)
        out[b, 1024 * hh:1024 * hh + 1024] = chunk
    return out


_LAST_RES = None


def kernel(**inputs):
    global _LAST_RES
    in_maps, fast = _in_maps(**inputs)
    nc = _get_nc(fast)
    res = run_bass_kernel_spmd(nc, in_maps, core_ids=list(range(8)))
    _LAST_RES = res
    return _assemble(res.results)


# revision 41
# speedup vs baseline: 1.0272x; 1.0272x over previous
"""Trainium2 Bass kernel for nn_EnhancedTransformerBlock_51917564674691.

Reference block (B=4, S=2048, D=256):
  x_global = global_mha(x, 8 heads, hd=32)          # dense S x S attention
  x_local  = local_mha(x, 4 heads, hd=64, window=5) # banded attention
  x_fused  = MLP_512(silu) over concat([x_global, x_local])
  x        = LN(x + x_fused); x = LN(x + FFN(x)); return x

Sharding: 8 cores = 4 batches x 2 sequence-halves. Each core computes the
full-batch K/V for global attention (needs all 2048 keys) and produces the
output for its 1024 tokens.

Layouts: "T-layout" = [feature partitions, token free] throughout the matmul
chain; host pre-transposes x and all weights into SBUF-image form so every
DMA is a contiguous [128, F] block. Attention internals are bf16 (fp32 PSUM
accumulation everywhere).

Pipelining: the global-attention score PSUM is split into two [128,1024]
half-tiles (2 banks each, double-buffered) so the PE score matmuls of key
tile kt+1 overlap the softmax exp of key tile kt on the scalar engine; the
AV accumulation for the second half lags one iteration so the PE never
waits on an in-flight exp. Softmax denominators come from ones-columns in
the V weights; divisions use the fast DVE reciprocal approximation.
Elementwise work is spread across Vector, GpSimd(Pool) and Scalar engines.
"""
import os
import numpy as np
import ml_dtypes

import concourse.bass as bass
import concourse.tile as tile
from concourse import bacc, mybir
from concourse.bass_utils import run_bass_kernel_spmd
from concourse.masks import make_identity

P = 128
BF = mybir.dt.bfloat16
F32 = mybir.dt.float32
FP8 = mybir.dt.float8e4
I8 = mybir.dt.int8
BF_NP = ml_dtypes.bfloat16

B, S, D = 4, 2048, 256
TQ = 1024           # tokens per core
XQ = 1152           # padded x_q length (own tokens + halo, zero padded)
NQT = 2             # global q tiles of 512
NKT = 16            # global key tiles of 128
GSC = 1.0 / np.sqrt(32.0)   # global attention scale
LSC = 0.125                 # local attention scale (1/sqrt(64))
# Schraudolph fast-exp constants targeting a bf16 bit pattern:
# bf16bits(exp(GSC*x)) ~= round(x * 128*GSC*log2(e) + (127*128 - 5.5))
A16 = float(128.0 * GSC * np.log2(np.e))
B16 = 16250.5
LB = 124            # local block queries
NLB = 9             # local blocks (9*124 = 1116 >= 1024)
EPS = 1e-5

AF = mybir.ActivationFunctionType
TT = mybir.AluOpType

# small per-partition tensors packed into one f32 image (name -> width)
PACKF = [
    ("bgq", 2), ("bgk", 2), ("btqk", 4), ("bgo", 2), ("bto", 2),
    ("bf1", 4), ("nbf1", 4), ("bn1", 4), ("nbn1", 4),
    ("bgv128", 256), ("btv128", 256), ("bn2128", 256),
    ("gng128", 256), ("gnb128", 256), ("fng128", 256), ("fnb128", 256),
]
PACKF_W = sum(w for _, w in PACKF)
PACKB = [("bandA", LB), ("bandF", LB), ("bandL", LB)]
PACKB_W = sum(w for _, w in PACKB)

# name -> (shape, np dtype) of per-core DRAM inputs (all SBUF-image [128, F]),
# in DMA issue order (alternating between the two HW DGE queues)
INPUT_SPECS = {
    "xqT": ((P, 2 * XQ), BF_NP),      # x_q.T padded      (own + halo, T-layout)
    "wgq": ((P, 2 * 256), BF_NP),
    "wgk": ((P, 2 * 256), BF_NP),
    "xkvT": ((P, 2 * 2048), BF_NP),   # x[b].T            (full batch, T-layout)
    "wtqk": ((P, 2 * 512), BF_NP),
    "wgv": ((P, 2 * 256), BF_NP),
    "wtv": ((P, 2 * 256), BF_NP),
    "packB": ((P, PACKB_W), BF_NP),
    "packF": ((P, PACKF_W), np.float32),
    "wgo": ((P, 2 * 256), BF_NP),
    "wto": ((P, 2 * 256), BF_NP),
    "wf1": ((P, 4 * 512), BF_NP),
    "wf2": ((P, 4 * 256), BF_NP),
    "wn1": ((P, 2 * 512), BF_NP),
    "wn2": ((P, 4 * 256), BF_NP),
    "xownN": ((P, 8 * 256), BF_NP),   # x own tokens + fus_b2 (N-layout)
}


def _patch_act_tables():
    """Make Exp and Ln resolve to the combined natural_log_exp_and_others set
    so the table-load pass emits ONE load instead of thrashing between
    exp_and_others and natural_log (9 loads, ~2.7us each + ACT drains)."""
    import concourse.hw_specs as hs
    if getattr(hs, "_act_tables_patched", False):
        return
    orig = hs.get_activation_tables

    def patched(module_arch):
        t = dict(orig(module_arch))
        exp = mybir.ActivationFunctionType.Exp
        ln = mybir.ActivationFunctionType.Ln
        for name in ("exp_and_others", "exp_and_friends"):
            if name in t:
                t[name] = t[name] - {exp}
        if "natural_log" in t:
            t["natural_log"] = t["natural_log"] - {ln}
        return t

    hs.get_activation_tables = patched
    import concourse.bacc as bc
    bc.get_activation_tables = patched
    hs._act_tables_patched = True


def build(fast):
    _patch_act_tables()
    nc = bacc.Bacc("TRN2", target_bir_lowering=False, debug=False, num_devices=8)
    dram = {}
    for name, (shape, npdt) in INPUT_SPECS.items():
        dram[name] = nc.dram_tensor(
            name, list(shape), mybir.dt.from_np(np.dtype(npdt)), kind="ExternalInput"
        ).ap()
    out_dram = nc.dram_tensor("out", [P, 8 * 256], F32, kind="ExternalOutput").ap()

    with tile.TileContext(nc) as tc:
        _emit(nc, tc, dram, out_dram, fast)
    nc.compile()
    return nc


def _emit(nc, tc, dram, out_dram, fast):
    from contextlib import ExitStack
    ctx = ExitStack()

    cpool = ctx.enter_context(tc.tile_pool(name="const", bufs=1))
    wpool = ctx.enter_context(tc.tile_pool(name="work", bufs=1))
    spool = ctx.enter_context(tc.tile_pool(name="scratch", bufs=4))
    epool = ctx.enter_context(tc.tile_pool(name="exps", bufs=2))
    pp = ctx.enter_context(tc.tile_pool(name="ps", bufs=1, space="PSUM"))

    def _kernel_body():
            # ---- load constants / inputs --------------------------------------
            # split DMAs across the two HW DGE queues (SP + ACT) in dependency
            # order; small tensors arrive pre-packed as two images
            cin = {}
            for n, (name, (shape, npdt)) in enumerate(INPUT_SPECS.items()):
                t = cpool.tile(list(shape), mybir.dt.from_np(np.dtype(npdt)), tag=name)
                eng = nc.sync if n % 2 == 0 else nc.scalar
                eng.dma_start(t[:], dram[name])
                cin[name] = t[:]
            off = 0
            for name, wd in PACKF:
                cin[name] = cin["packF"][:, off:off + wd]
                off += wd
            off = 0
            for name, wd in PACKB:
                cin[name] = cin["packB"][:, off:off + wd]
                off += wd

            ones_bf = cpool.tile([P, 64], BF, tag="ones_bf")
            nc.vector.memset(ones_bf[:], 1.0)
            eps_col = cpool.tile([P, 1], F32, tag="eps_col")
            nc.vector.memset(eps_col[:], EPS)
            cin["eps1"] = eps_col[:]
            ident = cpool.tile([P, P], F32, tag="ident")
            make_identity(nc, ident[:])

            # reshaped views of inputs
            xkvT = cin["xkvT"].rearrange("p (k n) -> p k n", k=2)     # [128,2,2048] bf
            xqT = cin["xqT"].rearrange("p (k n) -> p k n", k=2)       # [128,2,1152]
            xownN = cin["xownN"].rearrange("p (t f) -> p t f", t=8)   # [128,8,256] bf
            w = {k: cin[k].rearrange("p (k2 n) -> p k2 n", k2=2)
                 for k in ("wgq", "wgk", "wgv", "wtqk", "wtv", "wgo", "wto", "wn1")}
            w["wf1"] = cin["wf1"].rearrange("p (k2 n) -> p k2 n", k2=4)
            w["wf2"] = cin["wf2"].rearrange("p (k2 n) -> p k2 n", k2=4)
            w["wn2"] = cin["wn2"].rearrange("p (k2 n) -> p k2 n", k2=4)

            # ---- persistent intermediates ------------------------------------
            qT = wpool.tile([P, 2, 1024], BF, tag="qT")
            kT = wpool.tile([P, 2, 2048], BF, tag="kT")
            v_aug = wpool.tile([P, NKT, 8, 64], BF, tag="v_aug")
            qkL = wpool.tile([P, 4, XQ], BF, tag="qkL")
            vL = wpool.tile([P, NLB, 256], BF, tag="vL")
            g_oT = wpool.tile([P, 2, 1024], BF, tag="g_oT")
            l_oT = wpool.tile([P, 2, 1024], BF, tag="l_oT")
            combT = wpool.tile([P, 4, 1024], BF, tag="combT")
            h1s = wpool.tile([P, 4, 1024], BF, tag="h1s")
            x1N = wpool.tile([P, 8, 256], F32, tag="x1N")
            x1T = wpool.tile([P, 2, 1024], BF, tag="x1T")
            h2s = wpool.tile([P, 4, 1024], BF, tag="h2s")
            out_sb = wpool.tile([P, 8, 256], F32, tag="out_sb")

            def ps_sc():
                return pp.tile([P, 1024], F32, tag="sc", bufs=2, name="ps_sc")

            def ps_av():
                return pp.tile([P, 512], F32, tag="av", bufs=2, name="ps_av")

            def ps_sm():
                return pp.tile([P, 512], F32, tag="sm", bufs=2, name="ps_sm")

            def bias_bc(name, m, n):
                return cin[name][:, m:m + 1].to_broadcast([P, n])

            # ---- qkv projections (global) ------------------------------------
            # qT over own tokens first (x_q rows 2..1026)
            for m in range(2):
                for nt in range(2):
                    pm = ps_sm()
                    for k in range(2):
                        nc.tensor.matmul(pm[:], w["wgq"][:, k, 128 * m:128 * m + 128],
                                         xqT[:, k, 2 + 512 * nt:2 + 512 * nt + 512],
                                         start=(k == 0), stop=(k == 1))
                    if fast:
                        nc.vector.tensor_copy(qT[:, m, 512 * nt:512 * nt + 512], pm[:])
                    else:
                        nc.scalar.activation(qT[:, m, 512 * nt:512 * nt + 512], pm[:],
                                             AF.Identity, bias=cin["bgq"][:, m:m + 1])
            # kT = Wk @ x^T over full batch
            for m in range(2):
                for nt in range(4):
                    pm = ps_sm()
                    for k in range(2):
                        nc.tensor.matmul(pm[:], w["wgk"][:, k, 128 * m:128 * m + 128],
                                         xkvT[:, k, 512 * nt:512 * nt + 512],
                                         start=(k == 0), stop=(k == 1))
                    if fast:
                        nc.vector.tensor_copy(kT[:, m, 512 * nt:512 * nt + 512], pm[:])
                    else:
                        nc.scalar.activation(kT[:, m, 512 * nt:512 * nt + 512], pm[:],
                                             AF.Identity, bias=cin["bgk"][:, m:m + 1])
            # ---- qkv projections (local) -------------------------------------
            for m in range(4):
                for nt in range(3):
                    pm = ps_sm()
                    for k in range(2):
                        nc.tensor.matmul(pm[:, 0:384], w["wtqk"][:, k, 128 * m:128 * m + 128],
                                         xqT[:, k, 384 * nt:384 * nt + 384],
                                         start=(k == 0), stop=(k == 1))
                    nc.scalar.activation(qkL[:, m, 384 * nt:384 * nt + 384], pm[:, 0:384],
                                         AF.Identity, bias=cin["btqk"][:, m:m + 1])
            # v (N-layout, augmented with ones columns): v[key, f] over full batch
            nc.vector.memset(v_aug[:, :, :, 32:64], 1.0)
            for mt in range(16):
                pm = ps_sm()
                for k in range(2):
                    nc.tensor.matmul(pm[:, 0:256], xkvT[:, k, 128 * mt:128 * mt + 128],
                                     w["wgv"][:, k, :], start=(k == 0), stop=(k == 1))
                dst = v_aug[:, mt, :, 0:32]
                src = pm[:, 0:256].rearrange("p (h d) -> p h d", h=8)
                if fast:
                    nc.vector.tensor_copy(dst, src)
                else:
                    nc.vector.tensor_tensor(
                        dst, src,
                        cin["bgv128"].rearrange("p (h d) -> p h d", h=8), TT.add)
            for blk in range(NLB):
                pm = ps_sm()
                for k in range(2):
                    nc.tensor.matmul(pm[:, 0:256], xqT[:, k, 124 * blk:124 * blk + 128],
                                     w["wtv"][:, k, :], start=(k == 0), stop=(k == 1))
                if fast:
                    nc.vector.tensor_copy(vL[:, blk, :], pm[:, 0:256])
                else:
                    nc.vector.tensor_tensor(vL[:, blk, :], pm[:, 0:256],
                                            cin["btv128"], TT.add)

            # ---- local attention ---------------------------------------------
            # (pipelined: score PSUM halves double-buffered, exp per half,
            #  band mask on Pool, fast reciprocal on DVE)
            for blk in range(NLB):
                k0 = 124 * blk
                q0 = 2 + 124 * blk
                qn = 32 if blk == NLB - 1 else LB  # valid queries in this block
                eloc = epool.tile([P, 4, LB], BF, tag="eloc", bufs=2)
                for half in range(2):
                    psc = ps_sc()
                    pv = psc[:].rearrange("p (k n) -> p k n", k=2)
                    for r in range(2):
                        l = 2 * half + r
                        nc.tensor.matmul(pv[:, r, 0:LB],
                                         qkL[64 * r:64 * r + 64, 2 + half, k0:k0 + 128],
                                         qkL[64 * r:64 * r + 64, half, q0:q0 + LB],
                                         start=True, stop=True, tile_position=(64 * r, 0))
                    nc.scalar.activation(eloc[:, 2 * half:2 * half + 2, :],
                                         pv[:, :, 0:LB], AF.Exp, scale=LSC)
                band = ("bandF" if blk == 0 else
                        ("bandL" if blk == NLB - 1 else "bandA"))
                nc.vector.tensor_tensor(eloc[:], eloc[:],
                                        cin[band][:, None, :].to_broadcast([P, 4, LB]),
                                        TT.mult)  # cin[band] is an AP slice of packB
                pav = [ps_av(), ps_av()]
                pde = [ps_sm(), ps_sm()]
                for l in range(4):
                    pr, c = l // 2, l % 2
                    nc.tensor.matmul(pav[pr][64 * c:64 * c + 64, 0:LB],
                                     vL[:, blk, 64 * l:64 * l + 64], eloc[:, l, :],
                                     start=True, stop=True, tile_position=(0, 64 * c))
                    nc.tensor.matmul(pde[pr][64 * c:64 * c + 64, 0:LB],
                                     ones_bf[:], eloc[:, l, :],
                                     start=True, stop=True, tile_position=(0, 64 * c))
                for pr in range(2):
                    rec = spool.tile([P, LB], F32, tag="lrec", bufs=2)
                    nc.vector.reciprocal_approx_fast(rec[:], pde[pr][:, 0:LB])
                    nc.vector.tensor_tensor(l_oT[:, pr, k0:k0 + qn], pav[pr][:, 0:qn],
                                            rec[:, 0:qn], TT.mult)

            # ---- per-chunk: global attention + MLP tail ----------------------
            for qt in range(NQT):
                qsl = slice(512 * qt, 512 * qt + 512)
                for hg in range(2):
                    pav = [ps_av(), ps_av()]
                    lag = None  # (eg tile, kt) for the lagged half-B AV matmuls

                    def av_half(p2, eg, kt):
                        for c in range(2):
                            h = 4 * hg + 2 * p2 + c
                            # skip_group_check: CoreSim's zero-region tracker is
                            # partition-blind (any two concurrent groups per bank
                            # conflict); HW has per-element has_written bits and
                            # the 64-offset dual-group pattern is exact on HW.
                            nc.tensor.matmul(pav[p2][64 * c:64 * c + 64, :],
                                             v_aug[:, kt, h, :],
                                             eg[:, 512 * c:512 * c + 512],
                                             start=(kt == 0), stop=(kt == NKT - 1),
                                             tile_position=(0, 64 * c),
                                             skip_group_check=True)

                    for kt in range(NKT):
                        pscA = ps_sc()
                        for r in range(2):
                            nc.tensor.matmul(
                                pscA[:, 512 * r:512 * r + 512],
                                kT[32 * r:32 * r + 32, hg, 128 * kt:128 * kt + 128],
                                qT[32 * r:32 * r + 32, hg, qsl],
                                start=True, stop=True, tile_position=(32 * r, 0))
                        egA = epool.tile([P, 1024], BF, tag="egA", bufs=2)
                        nc.scalar.activation(egA[:], pscA[:], AF.Exp, scale=GSC)
                        pscB = ps_sc()
                        for r in range(2):
                            hc = 2 + r
                            nc.tensor.matmul(
                                pscB[:, 512 * r:512 * r + 512],
                                kT[32 * hc:32 * hc + 32, hg, 128 * kt:128 * kt + 128],
                                qT[32 * hc:32 * hc + 32, hg, qsl],
                                start=True, stop=True, tile_position=(32 * hc, 0))
                        # Schraudolph fast exp on DVE: scale+shift the score so
                        # the rounded int16 IS the bf16 bit pattern of
                        # exp(GSC*score) (softmax ratio cancels the
                        # piecewise-linear mantissa error)
                        egB = epool.tile([P, 1024], mybir.dt.int16, tag="egB",
                                         bufs=2)
                        nc.vector.tensor_scalar(egB[:], pscB[:], A16, B16,
                                                TT.mult, TT.add)
                        # AV lags one kt behind so the PE never waits on an
                        # in-flight exp
                        if lag is not None:
                            av_half(0, lag[0][:], lag[2])
                            av_half(1, lag[1][:].bitcast(BF), lag[2])
                        lag = (egA, egB, kt)
                    av_half(0, lag[0][:], lag[2])
                    av_half(1, lag[1][:].bitcast(BF), lag[2])

                    # normalize: one fast reciprocal per accumulator (denominator
                    # rows 32:64 / 96:128; extra rows are unused garbage)
                    for p2 in range(2):
                        rec = spool.tile([P, 512], F32, tag="grec", bufs=2)
                        nc.vector.reciprocal_approx_fast(rec[:], pav[p2][:])
                        nc.vector.tensor_tensor(g_oT[64 * p2:64 * p2 + 32, hg, qsl],
                                                pav[p2][0:32, :], rec[32:64, :], TT.mult)
                        nc.vector.tensor_tensor(g_oT[64 * p2 + 32:64 * p2 + 64, hg, qsl],
                                                pav[p2][64:96, :], rec[96:128, :], TT.mult)

                # ---- MLP tail in two 256-query column pipelines --------------
                # (halves the serial latency of the out-proj -> gemm1 -> silu
                #  -> gemm2 -> LN -> transpose -> FFN chain; the second half
                #  trails the first by one stage)
                for uu in range(2):
                    qsu = slice(512 * qt + 256 * uu, 512 * qt + 256 * uu + 256)
                    for m in range(2):
                        pm = ps_sm()
                        for k in range(2):
                            nc.tensor.matmul(pm[:, 0:256],
                                             w["wgo"][:, k, 128 * m:128 * m + 128],
                                             g_oT[:, k, qsu],
                                             start=(k == 0), stop=(k == 1))
                        if fast:
                            nc.scalar.activation(combT[:, m, qsu], pm[:, 0:256],
                                                 AF.Identity)
                        else:
                            nc.vector.tensor_tensor(combT[:, m, qsu], pm[:, 0:256],
                                                    bias_bc("bgo", m, 256), TT.add)
                    for m in range(2):
                        pm = ps_sm()
                        for k in range(2):
                            nc.tensor.matmul(pm[:, 0:256],
                                             w["wto"][:, k, 128 * m:128 * m + 128],
                                             l_oT[:, k, qsu],
                                             start=(k == 0), stop=(k == 1))
                        if fast:
                            nc.scalar.activation(combT[:, 2 + m, qsu], pm[:, 0:256],
                                                 AF.Identity)
                        else:
                            nc.vector.tensor_tensor(combT[:, 2 + m, qsu], pm[:, 0:256],
                                                    bias_bc("bto", m, 256), TT.add)

                    # fused MLP gemm1 + silu
                    for m in range(4):
                        pm = ps_sm()
                        for k in range(4):
                            nc.tensor.matmul(pm[:, 0:256],
                                             w["wf1"][:, k, 128 * m:128 * m + 128],
                                             combT[:, k, qsu],
                                             start=(k == 0), stop=(k == 3))
                        _silu(nc, spool, h1s[:, m, qsu], pm,
                              cin["bf1"], cin["nbf1"], m, fast)

                    # fused MLP gemm2 (N-layout out) + residual + LN1
                    _g2_res_ln(nc, spool, pp, cin, qt, uu, h1s, w["wf2"], None,
                               xownN, "gn", x1N, x1N_src=None, fast=fast)

                    # transpose x1N pair -> x1T
                    for tt in (2 * uu, 2 * uu + 1):
                        ta = 4 * qt + tt
                        for fh in range(2):
                            ptr = ps_sm()
                            nc.tensor.transpose(ptr[:, 0:128],
                                                x1N[:, ta, 128 * fh:128 * fh + 128],
                                                ident[:])
                            nc.scalar.activation(x1T[:, fh, 128 * ta:128 * ta + 128],
                                                 ptr[:, 0:128], AF.Identity)

                    # FFN gemm1 + silu
                    for m in range(4):
                        pm = ps_sm()
                        for k in range(2):
                            nc.tensor.matmul(pm[:, 0:256],
                                             w["wn1"][:, k, 128 * m:128 * m + 128],
                                             x1T[:, k, qsu],
                                             start=(k == 0), stop=(k == 1))
                        _silu(nc, spool, h2s[:, m, qsu], pm,
                              cin["bn1"], cin["nbn1"], m, fast)

                    # FFN gemm2 + residual(x1N) + LN2 -> out_sb
                    _g2_res_ln(nc, spool, pp, cin, qt, uu, h2s, w["wn2"], "bn2128",
                               None, "fn", out_sb, x1N_src=x1N, fast=fast)

                    # store half-chunk
                    nc.sync.dma_start(
                        out_dram[:, 1024 * qt + 512 * uu:1024 * qt + 512 * uu + 512],
                        out_sb[:, 4 * qt + 2 * uu:4 * qt + 2 * uu + 2, :]
                        .rearrange("p t f -> p (t f)"))

    REPEAT = int(os.environ.get("KREPEAT", "1"))
    if REPEAT > 1:
        with tc.For_i(0, REPEAT, 1):
            _kernel_body()
    else:
        _kernel_body()
    ctx.close()


def _silu(nc, spool, out_ap, pm, b_t, nb_t, m, fast):
    """out = silu(pm + b) where b is per-partition bias column m.

    silu(y) = y / (1 + exp(-y)); exp on ACT, then +1, fast reciprocal and
    the final (pm + b) * r on DVE."""
    src = pm[:, 0:256]
    e = spool.tile([P, 256], F32, tag="se", name="se", bufs=2)
    bias = 0.0 if fast else nb_t[:, m:m + 1]
    nc.scalar.activation(e[:], src, AF.Exp, bias=bias, scale=-1.0)
    nc.vector.tensor_scalar_add(e[:], e[:], 1.0)
    r = spool.tile([P, 256], F32, tag="sr", name="sr", bufs=2)
    nc.vector.reciprocal_approx_fast(r[:], e[:])
    if fast:
        nc.vector.tensor_tensor(out_ap, src, r[:], TT.mult)
    else:
        nc.vector.scalar_tensor_tensor(out_ap, src, b_t[:, m:m + 1], r[:],
                                       TT.add, TT.mult)


def _g2_res_ln(nc, spool, pp, cin, qt, uu, hsrc, w2, b128_name, xownN, ln_prefix,
               dest, x1N_src, fast):
    """gemm2 (contract 512 -> 256, N-layout out) + bias + residual + layernorm
    for the token pair (2*uu, 2*uu+1) of chunk qt.

    residual = xownN[:, ta, :] (already includes fus_b2, host-folded) or
    x1N_src[:, ta, :] (+ b128 on device unless fast).
    dest[:, ta, :] = LN(res + gemm2_out [+ b128]) [* g128 + b128_ln]
    The (x - mu) * istd normalize runs as ONE scalar-engine activation with
    per-partition scale/bias; sums for mu/var come fused out of the two DVE
    scalar_tensor_tensor ops (accum_out)."""
    xrs = []
    mu_raw = spool.tile([P, 2], F32, tag="mu_raw", bufs=2)
    s2_raw = spool.tile([P, 2], F32, tag="s2_raw", bufs=2)
    for tt in range(2):
        ta = 4 * qt + 2 * uu + tt
        pm = pp.tile([P, 512], F32, tag="sm", bufs=2, name="ps_sm")
        for k in range(4):
            nc.tensor.matmul(pm[:, 0:256], hsrc[:, k, 128 * ta:128 * ta + 128],
                             w2[:, k, :], start=(k == 0), stop=(k == 3))
        res = xownN[:, ta, :] if xownN is not None else x1N_src[:, ta, :]
        src = pm[:, 0:256]
        if not fast and b128_name is not None:
            tb = spool.tile([P, 256], F32, tag="tb", name="tb", bufs=2)
            nc.vector.tensor_tensor(tb[:], src, cin[b128_name], TT.add)
            src = tb[:]
        xr = spool.tile([P, 256], F32, tag=f"xr{tt}", name="xr")
        nc.vector.scalar_tensor_tensor(xr[:], src, 0.0, res, TT.add, TT.add,
                                       accum_out=mu_raw[:, tt:tt + 1])
        sq = spool.tile([P, 256], F32, tag="sq", name="sq", bufs=2)
        nc.vector.scalar_tensor_tensor(sq[:], xr[:], 0.0, xr[:], TT.add, TT.mult,
                                       accum_out=s2_raw[:, tt:tt + 1])
        xrs.append(xr)
    # fused stats: var = s2/256 - (mu_raw/256)^2; istd = exp(-ln(var+eps)/2);
    # nmui = -mu*istd  (5 serial ops instead of 9 — this chain is the
    # latency-critical part of the exposed tail)
    var = spool.tile([P, 2], F32, tag="var", bufs=2)
    istd = spool.tile([P, 2], F32, tag="istd", bufs=2)
    nmui = spool.tile([P, 2], F32, tag="nmui", bufs=2)
    nc.vector.scalar_tensor_tensor(var[:], mu_raw[:], 1.0 / 65536.0, mu_raw[:],
                                   TT.mult, TT.mult)
    nc.vector.scalar_tensor_tensor(var[:], s2_raw[:], 1.0 / 256.0, var[:],
                                   TT.mult, TT.subtract)
    nc.scalar.activation(var[:], var[:], AF.Ln, bias=cin["eps1"])
    nc.scalar.activation(istd[:], var[:], AF.Exp, scale=-0.5)
    nc.vector.scalar_tensor_tensor(nmui[:], mu_raw[:], -1.0 / 256.0, istd[:],
                                   TT.mult, TT.mult)
    for tt in range(2):
        ta = 4 * qt + 2 * uu + tt
        xr = xrs[tt]
        if fast:
            nc.scalar.activation(dest[:, ta, :], xr[:], AF.Identity,
                                 bias=nmui[:, tt:tt + 1], scale=istd[:, tt:tt + 1])
        else:
            nc.scalar.activation(xr[:], xr[:], AF.Identity,
                                 bias=nmui[:, tt:tt + 1], scale=istd[:, tt:tt + 1])
            nc.vector.tensor_tensor(xr[:], xr[:], cin[ln_prefix + "g128"], TT.mult)
            nc.vector.tensor_tensor(dest[:, ta, :], xr[:], cin[ln_prefix + "b128"],
                                    TT.add)


# ======================================================================
# Host side
# ======================================================================

_NC = {}


def _get_nc(fast):
    if fast not in _NC:
        _NC[fast] = build(fast)
    return _NC[fast]


def _img_T(mat):
    """[R, C] fp32 (R = k*128) -> SBUF image [128, k*C] for T-layout tiles."""
    R, C = mat.shape
    k = R // 128
    return np.ascontiguousarray(
        mat.reshape(k, 128, C).transpose(1, 0, 2).reshape(128, k * C))


def _img_N(mat):
    """[T, F] (T = t*128) -> SBUF image [128, t*F] for N-layout tiles."""
    T, F = mat.shape
    t = T // 128
    return np.ascontiguousarray(
        mat.reshape(t, 128, F).transpose(1, 0, 2).reshape(128, t * F))


def _bias_cols(b):
    """[k*128] -> [128, k] per-partition column layout."""
    return np.ascontiguousarray(b.reshape(-1, 128).T)


def _in_maps(x, g_in_w, g_in_b, g_out_w, g_out_b,
             t_in_w, t_in_b, t_out_w, t_out_b,
             fus_w1, fus_b1, fus_w2, fus_b2,
             ffn_w1, ffn_b1, ffn_w2, ffn_b2,
             gn_g, gn_b, fn_g, fn_b):
    x = np.asarray(x, np.float32)
    f32 = lambda a: np.asarray(a, np.float32)
    bf = lambda a: np.asarray(a, np.float32).astype(BF_NP)

    # fast path: every remaining device-side bias is zero and LN affine
    # params are identity (guaranteed by the problem's input fills; the
    # generic path handles anything else)
    fast = bool(
        np.all(f32(g_in_b)[512:768] == 0) and np.all(f32(t_in_b)[512:768] == 0)
        and np.all(f32(g_out_b) == 0) and np.all(f32(t_out_b) == 0)
        and np.all(f32(ffn_b2) == 0)
        and np.all(f32(gn_g) == 1) and np.all(f32(gn_b) == 0)
        and np.all(f32(fn_g) == 1) and np.all(f32(fn_b) == 0))

    # shared (same on all cores) tensors
    shared = {
        "wgq": bf(_img_T(f32(g_in_w)[0:256].T)),
        "wgk": bf(_img_T(f32(g_in_w)[256:512].T)),
        "wgv": bf(_img_T(f32(g_in_w)[512:768].T)),
        "wtqk": bf(_img_T(f32(t_in_w)[0:512].T)),
        "wtv": bf(_img_T(f32(t_in_w)[512:768].T)),
        "wgo": bf(_img_T(f32(g_out_w).T)),
        "wto": bf(_img_T(f32(t_out_w).T)),
        "wf1": bf(_img_T(f32(fus_w1).T)),
        "wf2": bf(_img_T(f32(fus_w2).T)),
        "wn1": bf(_img_T(f32(ffn_w1).T)),
        "wn2": bf(_img_T(f32(ffn_w2).T)),
    }
    packs = {
        "bgq": _bias_cols(f32(g_in_b)[0:256]),
        "bgk": _bias_cols(f32(g_in_b)[256:512]),
        "btqk": _bias_cols(f32(t_in_b)[0:512]),
        "bgo": _bias_cols(f32(g_out_b)),
        "bto": _bias_cols(f32(t_out_b)),
        "bf1": _bias_cols(f32(fus_b1)),
        "nbf1": _bias_cols(-f32(fus_b1)),
        "bn1": _bias_cols(f32(ffn_b1)),
        "nbn1": _bias_cols(-f32(ffn_b1)),
        "bgv128": np.broadcast_to(f32(g_in_b)[512:768], (P, 256)),
        "btv128": np.broadcast_to(f32(t_in_b)[512:768], (P, 256)),
        "bn2128": np.broadcast_to(f32(ffn_b2), (P, 256)),
        "gng128": np.broadcast_to(f32(gn_g), (P, 256)),
        "gnb128": np.broadcast_to(f32(gn_b), (P, 256)),
        "fng128": np.broadcast_to(f32(fn_g), (P, 256)),
        "fnb128": np.broadcast_to(f32(fn_b), (P, 256)),
    }
    shared["packF"] = np.ascontiguousarray(
        np.concatenate([packs[n] for n, _ in PACKF], axis=1).astype(np.float32))
    # band mask: key row j valid for query qq iff qq <= j <= qq+4
    jj = np.arange(P)[:, None]
    qq = np.arange(LB)[None, :]
    bandA = ((qq <= jj) & (jj <= qq + 4)).astype(np.float32)
    bandF = bandA.copy()
    bandF[0:2] = 0.0           # keys at tokens -2, -1 (first block, first half)
    bandL = bandA.copy()
    bandL[34:36] = 0.0         # block-8 keys x_q rows 1026, 1027 (= S, S+1)

    in_maps = []
    for c in range(8):
        b, hh = c // 2, c % 2
        t0 = 1024 * hh
        xb = x[b]                                    # [2048, 256]
        xq = np.zeros((XQ + 4, D), np.float32)       # rows = x_q tokens t0-2 ..
        lo, hi = max(0, t0 - 2), min(S, t0 + XQ + 2)
        xq[lo - (t0 - 2):hi - (t0 - 2)] = xb[lo:hi]
        xq = xq[:XQ]                                 # guard: only XQ rows used
        m = dict(shared)
        m["xkvT"] = bf(_img_T(xb.T))
        m["xqT"] = bf(_img_T(xq.T))
        m["xownN"] = bf(_img_N(xb[t0:t0 + 1024] + f32(fus_b2)[None, :]))
        m["packB"] = np.ascontiguousarray(np.concatenate(
            [bandA, bandF if hh == 0 else bandA, bandL if hh == 1 else bandA],
            axis=1)).astype(BF_NP)
        in_maps.append(m)
    return in_maps, fast


def _assemble(results):
    out = np.zeros((B, S, D), np.float32)
    for c in range(8):
        b, hh = c // 2, c % 2
        img = results[c]["out"]                      # [128, 2048]
        chunk = img.reshape(P, 8, 256).transpose(1, 0, 2).reshape(1024, 256)
        out[b, 1024 * hh:1024 * hh + 1024] = chunk
    return out


_LAST_RES = None


def kernel(**inputs):
    global _LAST_RES
    in_maps, fast = _in_maps(**inputs)
    nc = _get_nc(fast)
    res = run_bass_kernel_spmd(nc, in_maps, core_ids=list(range(8)))
    _LAST_RES = res
    return _assemble(res.results)


# revision 43
# speedup vs baseline: 1.0692x; 1.0409x over previous
"""Trainium2 Bass kernel for nn_EnhancedTransformerBlock_51917564674691.

Reference block (B=4, S=2048, D=256):
  x_global = global_mha(x, 8 heads, hd=32)          # dense S x S attention
  x_local  = local_mha(x, 4 heads, hd=64, window=5) # banded attention
  x_fused  = MLP_512(silu) over concat([x_global, x_local])
  x        = LN(x + x_fused); x = LN(x + FFN(x)); return x

Sharding: 8 cores = 4 batches x 2 sequence-halves. Each core computes the
full-batch K/V for global attention (needs all 2048 keys) and produces the
output for its 1024 tokens.

Layouts: "T-layout" = [feature partitions, token free] throughout the matmul
chain; host pre-transposes x and all weights into SBUF-image form so every
DMA is a contiguous [128, F] block. Attention internals are bf16 (fp32 PSUM
accumulation everywhere).

Pipelining: the global-attention score PSUM is split into two [128,1024]
half-tiles (2 banks each, double-buffered) so the PE score matmuls of key
tile kt+1 overlap the softmax exp of key tile kt on the scalar engine; the
AV accumulation for the second half lags one iteration so the PE never
waits on an in-flight exp. Softmax denominators come from ones-columns in
the V weights; divisions use the fast DVE reciprocal approximation.
Elementwise work is spread across Vector, GpSimd(Pool) and Scalar engines.
"""
import os
import numpy as np
import ml_dtypes

import concourse.bass as bass
import concourse.tile as tile
from concourse import bacc, mybir
from concourse.bass_utils import run_bass_kernel_spmd
from concourse.masks import make_identity

P = 128
BF = mybir.dt.bfloat16
F32 = mybir.dt.float32
FP8 = mybir.dt.float8e4
I8 = mybir.dt.int8
BF_NP = ml_dtypes.bfloat16

B, S, D = 4, 2048, 256
TQ = 1024           # tokens per core
XQ = 1152           # padded x_q length (own tokens + halo, zero padded)
NQT = 2             # global q tiles of 512
NKT = 16            # global key tiles of 128
GSC = 1.0 / np.sqrt(32.0)   # global attention scale
LSC = 0.125                 # local attention scale (1/sqrt(64))
# Schraudolph fast-exp constants targeting a bf16 bit pattern:
# bf16bits(exp(GSC*x)) ~= round(x * 128*GSC*log2(e) + (127*128 - 5.5))
A16 = float(128.0 * GSC * np.log2(np.e))
B16 = 16250.5
LB = 124            # local block queries
NLB = 9             # local blocks (9*124 = 1116 >= 1024)
EPS = 1e-5

AF = mybir.ActivationFunctionType
TT = mybir.AluOpType

# small per-partition tensors packed into one f32 image (name -> width)
PACKF = [
    ("bgq", 2), ("bgk", 2), ("btqk", 4), ("bgo", 2), ("bto", 2),
    ("bf1", 4), ("nbf1", 4), ("bn1", 4), ("nbn1", 4),
    ("bgv128", 256), ("btv128", 256), ("bn2128", 256),
    ("gng128", 256), ("gnb128", 256), ("fng128", 256), ("fnb128", 256),
]
PACKF_W = sum(w for _, w in PACKF)
PACKB = [("bandA", LB), ("bandF", LB), ("bandL", LB)]
PACKB_W = sum(w for _, w in PACKB)

# name -> (shape, np dtype) of per-core DRAM inputs (all SBUF-image [128, F]),
# in DMA issue order (alternating between the two HW DGE queues)
INPUT_SPECS = {
    "xqT": ((P, 2 * XQ), BF_NP),      # x_q.T padded      (own + halo, T-layout)
    "wgq": ((P, 2 * 256), BF_NP),
    "wgk": ((P, 2 * 256), BF_NP),
    "xkvT": ((P, 2 * 2048), BF_NP),   # x[b].T            (full batch, T-layout)
    "wtqk": ((P, 2 * 512), BF_NP),
    "wgv": ((P, 2 * 256), BF_NP),
    "wtv": ((P, 2 * 256), BF_NP),
    "packB": ((P, PACKB_W), BF_NP),
    "packF": ((P, PACKF_W), np.float32),
    "wgo": ((P, 2 * 256), BF_NP),
    "wto": ((P, 2 * 256), BF_NP),
    "wf1": ((P, 4 * 512), BF_NP),
    "wf2": ((P, 4 * 256), BF_NP),
    "wn1": ((P, 2 * 512), BF_NP),
    "wn2": ((P, 4 * 256), BF_NP),
    "xownN": ((P, 8 * 256), BF_NP),   # x own tokens + fus_b2 (N-layout)
}


def _patch_act_tables():
    """Make Exp and Ln resolve to the combined natural_log_exp_and_others set
    so the table-load pass emits ONE load instead of thrashing between
    exp_and_others and natural_log (9 loads, ~2.7us each + ACT drains)."""
    import concourse.hw_specs as hs
    if getattr(hs, "_act_tables_patched", False):
        return
    orig = hs.get_activation_tables

    def patched(module_arch):
        t = dict(orig(module_arch))
        exp = mybir.ActivationFunctionType.Exp
        ln = mybir.ActivationFunctionType.Ln
        for name in ("exp_and_others", "exp_and_friends"):
            if name in t:
                t[name] = t[name] - {exp}
        if "natural_log" in t:
            t["natural_log"] = t["natural_log"] - {ln}
        return t

    hs.get_activation_tables = patched
    import concourse.bacc as bc
    bc.get_activation_tables = patched
    hs._act_tables_patched = True


def build(fast):
    _patch_act_tables()
    nc = bacc.Bacc("TRN2", target_bir_lowering=False, debug=False, num_devices=8)
    dram = {}
    for name, (shape, npdt) in INPUT_SPECS.items():
        dram[name] = nc.dram_tensor(
            name, list(shape), mybir.dt.from_np(np.dtype(npdt)), kind="ExternalInput"
        ).ap()
    out_dram = nc.dram_tensor("out", [P, 8 * 256], F32, kind="ExternalOutput").ap()

    with tile.TileContext(nc) as tc:
        _emit(nc, tc, dram, out_dram, fast)
    nc.compile()
    return nc


def _emit(nc, tc, dram, out_dram, fast):
    from contextlib import ExitStack
    ctx = ExitStack()

    cpool = ctx.enter_context(tc.tile_pool(name="const", bufs=1))
    wpool = ctx.enter_context(tc.tile_pool(name="work", bufs=1))
    spool = ctx.enter_context(tc.tile_pool(name="scratch", bufs=4))
    epool = ctx.enter_context(tc.tile_pool(name="exps", bufs=2))
    pp = ctx.enter_context(tc.tile_pool(name="ps", bufs=1, space="PSUM"))

    def _kernel_body():
            # ---- load constants / inputs --------------------------------------
            # split DMAs across the two HW DGE queues (SP + ACT) in dependency
            # order; small tensors arrive pre-packed as two images
            cin = {}
            for n, (name, (shape, npdt)) in enumerate(INPUT_SPECS.items()):
                t = cpool.tile(list(shape), mybir.dt.from_np(np.dtype(npdt)), tag=name)
                eng = nc.sync if n % 2 == 0 else nc.scalar
                eng.dma_start(t[:], dram[name])
                cin[name] = t[:]
            off = 0
            for name, wd in PACKF:
                cin[name] = cin["packF"][:, off:off + wd]
                off += wd
            off = 0
            for name, wd in PACKB:
                cin[name] = cin["packB"][:, off:off + wd]
                off += wd

            ones_bf = cpool.tile([P, 64], BF, tag="ones_bf")
            nc.vector.memset(ones_bf[:], 1.0)
            eps_col = cpool.tile([P, 1], F32, tag="eps_col")
            nc.vector.memset(eps_col[:], EPS)
            cin["eps1"] = eps_col[:]
            ident = cpool.tile([P, P], F32, tag="ident")
            make_identity(nc, ident[:])

            # reshaped views of inputs
            xkvT = cin["xkvT"].rearrange("p (k n) -> p k n", k=2)     # [128,2,2048] bf
            xqT = cin["xqT"].rearrange("p (k n) -> p k n", k=2)       # [128,2,1152]
            xownN = cin["xownN"].rearrange("p (t f) -> p t f", t=8)   # [128,8,256] bf
            w = {k: cin[k].rearrange("p (k2 n) -> p k2 n", k2=2)
                 for k in ("wgq", "wgk", "wgv", "wtqk", "wtv", "wgo", "wto", "wn1")}
            w["wf1"] = cin["wf1"].rearrange("p (k2 n) -> p k2 n", k2=4)
            w["wf2"] = cin["wf2"].rearrange("p (k2 n) -> p k2 n", k2=4)
            w["wn2"] = cin["wn2"].rearrange("p (k2 n) -> p k2 n", k2=4)

            # ---- persistent intermediates ------------------------------------
            qT = wpool.tile([P, 2, 1024], BF, tag="qT")
            kT = wpool.tile([P, 2, 2048], BF, tag="kT")
            v_aug = wpool.tile([P, NKT, 8, 64], BF, tag="v_aug")
            qkL = wpool.tile([P, 4, XQ], BF, tag="qkL")
            vL = wpool.tile([P, NLB, 256], BF, tag="vL")
            g_oT = wpool.tile([P, 2, 1024], BF, tag="g_oT")
            l_oT = wpool.tile([P, 2, 1024], BF, tag="l_oT")
            combT = wpool.tile([P, 4, 1024], BF, tag="combT")
            h1s = wpool.tile([P, 4, 1024], BF, tag="h1s")
            x1N = wpool.tile([P, 8, 256], F32, tag="x1N")
            x1T = wpool.tile([P, 2, 1024], BF, tag="x1T")
            h2s = wpool.tile([P, 4, 1024], BF, tag="h2s")
            out_sb = wpool.tile([P, 8, 256], F32, tag="out_sb")

            def ps_sc():
                return pp.tile([P, 1024], F32, tag="sc", bufs=2, name="ps_sc")

            def ps_av():
                return pp.tile([P, 512], F32, tag="av", bufs=2, name="ps_av")

            def ps_sm():
                return pp.tile([P, 512], F32, tag="sm", bufs=2, name="ps_sm")

            def bias_bc(name, m, n):
                return cin[name][:, m:m + 1].to_broadcast([P, n])

            # ---- qkv projections (global) ------------------------------------
            # qT over own tokens first (x_q rows 2..1026)
            for m in range(2):
                for nt in range(2):
                    pm = ps_sm()
                    for k in range(2):
                        nc.tensor.matmul(pm[:], w["wgq"][:, k, 128 * m:128 * m + 128],
                                         xqT[:, k, 2 + 512 * nt:2 + 512 * nt + 512],
                                         start=(k == 0), stop=(k == 1))
                    if fast:
                        nc.vector.tensor_copy(qT[:, m, 512 * nt:512 * nt + 512], pm[:])
                    else:
                        nc.scalar.activation(qT[:, m, 512 * nt:512 * nt + 512], pm[:],
                                             AF.Identity, bias=cin["bgq"][:, m:m + 1])
            # kT = Wk @ x^T over full batch
            for m in range(2):
                for nt in range(4):
                    pm = ps_sm()
                    for k in range(2):
                        nc.tensor.matmul(pm[:], w["wgk"][:, k, 128 * m:128 * m + 128],
                                         xkvT[:, k, 512 * nt:512 * nt + 512],
                                         start=(k == 0), stop=(k == 1))
                    if fast:
                        nc.vector.tensor_copy(kT[:, m, 512 * nt:512 * nt + 512], pm[:])
                    else:
                        nc.scalar.activation(kT[:, m, 512 * nt:512 * nt + 512], pm[:],
                                             AF.Identity, bias=cin["bgk"][:, m:m + 1])
            # ---- qkv projections (local) -------------------------------------
            for m in range(4):
                for nt in range(3):
                    pm = ps_sm()
                    for k in range(2):
                        nc.tensor.matmul(pm[:, 0:384], w["wtqk"][:, k, 128 * m:128 * m + 128],
                                         xqT[:, k, 384 * nt:384 * nt + 384],
                                         start=(k == 0), stop=(k == 1))
                    nc.scalar.activation(qkL[:, m, 384 * nt:384 * nt + 384], pm[:, 0:384],
                                         AF.Identity, bias=cin["btqk"][:, m:m + 1])
            # v (N-layout, augmented with ones columns): v[key, f] over full batch
            nc.vector.memset(v_aug[:, :, :, 32:64], 1.0)
            for mt in range(16):
                pm = ps_sm()
                for k in range(2):
                    nc.tensor.matmul(pm[:, 0:256], xkvT[:, k, 128 * mt:128 * mt + 128],
                                     w["wgv"][:, k, :], start=(k == 0), stop=(k == 1))
                dst = v_aug[:, mt, :, 0:32]
                src = pm[:, 0:256].rearrange("p (h d) -> p h d", h=8)
                if fast:
                    nc.vector.tensor_copy(dst, src)
                else:
                    nc.vector.tensor_tensor(
                        dst, src,
                        cin["bgv128"].rearrange("p (h d) -> p h d", h=8), TT.add)
            for blk in range(NLB):
                pm = ps_sm()
                for k in range(2):
                    nc.tensor.matmul(pm[:, 0:256], xqT[:, k, 124 * blk:124 * blk + 128],
                                     w["wtv"][:, k, :], start=(k == 0), stop=(k == 1))
                if fast:
                    nc.vector.tensor_copy(vL[:, blk, :], pm[:, 0:256])
                else:
                    nc.vector.tensor_tensor(vL[:, blk, :], pm[:, 0:256],
                                            cin["btv128"], TT.add)

            # ---- local attention ---------------------------------------------
            # (pipelined: score PSUM halves double-buffered, exp per half,
            #  band mask on Pool, fast reciprocal on DVE)
            for blk in range(NLB):
                k0 = 124 * blk
                q0 = 2 + 124 * blk
                qn = 32 if blk == NLB - 1 else LB  # valid queries in this block
                eloc = epool.tile([P, 4, LB], BF, tag="eloc", bufs=2)
                for half in range(2):
                    psc = ps_sc()
                    pv = psc[:].rearrange("p (k n) -> p k n", k=2)
                    for r in range(2):
                        l = 2 * half + r
                        nc.tensor.matmul(pv[:, r, 0:LB],
                                         qkL[64 * r:64 * r + 64, 2 + half, k0:k0 + 128],
                                         qkL[64 * r:64 * r + 64, half, q0:q0 + LB],
                                         start=True, stop=True, tile_position=(64 * r, 0))
                    nc.scalar.activation(eloc[:, 2 * half:2 * half + 2, :],
                                         pv[:, :, 0:LB], AF.Exp, scale=LSC)
                band = ("bandF" if blk == 0 else
                        ("bandL" if blk == NLB - 1 else "bandA"))
                nc.vector.tensor_tensor(eloc[:], eloc[:],
                                        cin[band][:, None, :].to_broadcast([P, 4, LB]),
                                        TT.mult)  # cin[band] is an AP slice of packB
                pav = [ps_av(), ps_av()]
                pde = [ps_sm(), ps_sm()]
                for l in range(4):
                    pr, c = l // 2, l % 2
                    nc.tensor.matmul(pav[pr][64 * c:64 * c + 64, 0:LB],
                                     vL[:, blk, 64 * l:64 * l + 64], eloc[:, l, :],
                                     start=True, stop=True, tile_position=(0, 64 * c))
                    nc.tensor.matmul(pde[pr][64 * c:64 * c + 64, 0:LB],
                                     ones_bf[:], eloc[:, l, :],
                                     start=True, stop=True, tile_position=(0, 64 * c))
                for pr in range(2):
                    rec = spool.tile([P, LB], F32, tag="lrec", bufs=2)
                    nc.vector.reciprocal_approx_fast(rec[:], pde[pr][:, 0:LB])
                    nc.vector.tensor_tensor(l_oT[:, pr, k0:k0 + qn], pav[pr][:, 0:qn],
                                            rec[:, 0:qn], TT.mult)

            # ---- per-chunk: global attention + MLP tail ----------------------
            for qt in range(NQT):
                qsl = slice(512 * qt, 512 * qt + 512)
                for hg in range(2):
                    pav = [ps_av(), ps_av()]
                    lag = None  # (eg tile, kt) for the lagged half-B AV matmuls

                    def av_half(p2, eg, kt):
                        for c in range(2):
                            h = 4 * hg + 2 * p2 + c
                            # skip_group_check: CoreSim's zero-region tracker is
                            # partition-blind (any two concurrent groups per bank
                            # conflict); HW has per-element has_written bits and
                            # the 64-offset dual-group pattern is exact on HW.
                            nc.tensor.matmul(pav[p2][64 * c:64 * c + 64, :],
                                             v_aug[:, kt, h, :],
                                             eg[:, 512 * c:512 * c + 512],
                                             start=(kt == 0), stop=(kt == NKT - 1),
                                             tile_position=(0, 64 * c),
                                             skip_group_check=True)

                    for kt in range(NKT):
                        pscA = ps_sc()
                        for r in range(2):
                            nc.tensor.matmul(
                                pscA[:, 512 * r:512 * r + 512],
                                kT[32 * r:32 * r + 32, hg, 128 * kt:128 * kt + 128],
                                qT[32 * r:32 * r + 32, hg, qsl],
                                start=True, stop=True, tile_position=(32 * r, 0))
                        egA = epool.tile([P, 1024], BF, tag="egA", bufs=2)
                        nc.scalar.activation(egA[:], pscA[:], AF.Exp, scale=GSC)
                        pscB = ps_sc()
                        for r in range(2):
                            hc = 2 + r
                            nc.tensor.matmul(
                                pscB[:, 512 * r:512 * r + 512],
                                kT[32 * hc:32 * hc + 32, hg, 128 * kt:128 * kt + 128],
                                qT[32 * hc:32 * hc + 32, hg, qsl],
                                start=True, stop=True, tile_position=(32 * hc, 0))
                        # Schraudolph fast exp on DVE: scale+shift the score so
                        # the rounded int16 IS the bf16 bit pattern of
                        # exp(GSC*score) (softmax ratio cancels the
                        # piecewise-linear mantissa error)
                        egB = epool.tile([P, 1024], mybir.dt.int16, tag="egB",
                                         bufs=2)
                        nc.vector.tensor_scalar(egB[:], pscB[:], A16, B16,
                                                TT.mult, TT.add)
                        # AV lags one kt behind so the PE never waits on an
                        # in-flight exp
                        if lag is not None:
                            av_half(0, lag[0][:], lag[2])
                            av_half(1, lag[1][:].bitcast(BF), lag[2])
                        lag = (egA, egB, kt)
                    av_half(0, lag[0][:], lag[2])
                    av_half(1, lag[1][:].bitcast(BF), lag[2])

                    # normalize: one fast reciprocal per accumulator (denominator
                    # rows 32:64 / 96:128; extra rows are unused garbage)
                    for p2 in range(2):
                        rec = spool.tile([P, 512], F32, tag="grec", bufs=2)
                        nc.vector.reciprocal_approx_fast(rec[:], pav[p2][:])
                        nc.vector.tensor_tensor(g_oT[64 * p2:64 * p2 + 32, hg, qsl],
                                                pav[p2][0:32, :], rec[32:64, :], TT.mult)
                        nc.vector.tensor_tensor(g_oT[64 * p2 + 32:64 * p2 + 64, hg, qsl],
                                                pav[p2][64:96, :], rec[96:128, :], TT.mult)

                # ---- out projections -> combT --------------------------------
                for m in range(2):
                    pm = ps_sm()
                    for k in range(2):
                        nc.tensor.matmul(pm[:], w["wgo"][:, k, 128 * m:128 * m + 128],
                                         g_oT[:, k, qsl],
                                         start=(k == 0), stop=(k == 1))
                    if fast:
                        nc.scalar.activation(combT[:, m, qsl], pm[:], AF.Identity)
                    else:
                        nc.vector.tensor_tensor(combT[:, m, qsl], pm[:],
                                                bias_bc("bgo", m, 512), TT.add)
                for m in range(2):
                    pm = ps_sm()
                    for k in range(2):
                        nc.tensor.matmul(pm[:], w["wto"][:, k, 128 * m:128 * m + 128],
                                         l_oT[:, k, qsl],
                                         start=(k == 0), stop=(k == 1))
                    if fast:
                        nc.scalar.activation(combT[:, 2 + m, qsl], pm[:], AF.Identity)
                    else:
                        nc.vector.tensor_tensor(combT[:, 2 + m, qsl], pm[:],
                                                bias_bc("bto", m, 512), TT.add)

                # ---- fused MLP gemm1 + silu ----------------------------------
                for m in range(4):
                    pm = ps_sm()
                    for k in range(4):
                        nc.tensor.matmul(pm[:], w["wf1"][:, k, 128 * m:128 * m + 128],
                                         combT[:, k, qsl],
                                         start=(k == 0), stop=(k == 3))
                    _silu(nc, spool, h1s[:, m, qsl], pm,
                          cin["bf1"], cin["nbf1"], m, fast)

                # fused MLP gemm2 + residual + LN1, per token pair (the LN
                # stats chain of pair 0 overlaps the gemms of pair 1), with
                # the transposes for each pair chased immediately
                for uu in range(2):
                    _g2_res_ln(nc, spool, pp, cin, qt, uu, h1s, w["wf2"], None,
                               xownN, "gn", x1N, x1N_src=None, fast=fast)
                    for tt in (2 * uu, 2 * uu + 1):
                        ta = 4 * qt + tt
                        for fh in range(2):
                            ptr = ps_sm()
                            nc.tensor.transpose(ptr[:, 0:128],
                                                x1N[:, ta, 128 * fh:128 * fh + 128],
                                                ident[:])
                            nc.scalar.activation(x1T[:, fh, 128 * ta:128 * ta + 128],
                                                 ptr[:, 0:128], AF.Identity)

                # FFN gemm1 + silu
                for m in range(4):
                    pm = ps_sm()
                    for k in range(2):
                        nc.tensor.matmul(pm[:], w["wn1"][:, k, 128 * m:128 * m + 128],
                                         x1T[:, k, qsl],
                                         start=(k == 0), stop=(k == 1))
                    _silu(nc, spool, h2s[:, m, qsl], pm,
                          cin["bn1"], cin["nbn1"], m, fast)

                # FFN gemm2 + residual(x1N) + LN2 -> out_sb, per token pair
                for uu in range(2):
                    _g2_res_ln(nc, spool, pp, cin, qt, uu, h2s, w["wn2"], "bn2128",
                               None, "fn", out_sb, x1N_src=x1N, fast=fast)
                    nc.sync.dma_start(
                        out_dram[:, 1024 * qt + 512 * uu:1024 * qt + 512 * uu + 512],
                        out_sb[:, 4 * qt + 2 * uu:4 * qt + 2 * uu + 2, :]
                        .rearrange("p t f -> p (t f)"))

    REPEAT = int(os.environ.get("KREPEAT", "1"))
    if REPEAT > 1:
        with tc.For_i(0, REPEAT, 1):
            _kernel_body()
    else:
        _kernel_body()
    ctx.close()


def _silu(nc, spool, out_ap, pm, b_t, nb_t, m, fast):
    """out = silu(pm + b) where b is per-partition bias column m.

    silu(y) = y / (1 + exp(-y)); exp on ACT, then +1, fast reciprocal and
    the final (pm + b) * r on DVE."""
    src = pm[:]
    e = spool.tile([P, 512], F32, tag="se", name="se", bufs=2)
    bias = 0.0 if fast else nb_t[:, m:m + 1]
    nc.scalar.activation(e[:], src, AF.Exp, bias=bias, scale=-1.0)
    nc.vector.tensor_scalar_add(e[:], e[:], 1.0)
    r = spool.tile([P, 512], F32, tag="sr", name="sr", bufs=2)
    nc.vector.reciprocal_approx_fast(r[:], e[:])
    if fast:
        nc.vector.tensor_tensor(out_ap, src, r[:], TT.mult)
    else:
        nc.vector.scalar_tensor_tensor(out_ap, src, b_t[:, m:m + 1], r[:],
                                       TT.add, TT.mult)


def _g2_res_ln(nc, spool, pp, cin, qt, uu, hsrc, w2, b128_name, xownN, ln_prefix,
               dest, x1N_src, fast):
    """gemm2 (contract 512 -> 256, N-layout out) + bias + residual + layernorm
    for the token pair (2*uu, 2*uu+1) of chunk qt.

    residual = xownN[:, ta, :] (already includes fus_b2, host-folded) or
    x1N_src[:, ta, :] (+ b128 on device unless fast).
    dest[:, ta, :] = LN(res + gemm2_out [+ b128]) [* g128 + b128_ln]
    The (x - mu) * istd normalize runs as ONE scalar-engine activation with
    per-partition scale/bias; sums for mu/var come fused out of the two DVE
    scalar_tensor_tensor ops (accum_out)."""
    xrs = []
    mu_raw = spool.tile([P, 2], F32, tag="mu_raw", bufs=2)
    s2_raw = spool.tile([P, 2], F32, tag="s2_raw", bufs=2)
    for tt in range(2):
        ta = 4 * qt + 2 * uu + tt
        pm = pp.tile([P, 512], F32, tag="sm", bufs=2, name="ps_sm")
        for k in range(4):
            nc.tensor.matmul(pm[:, 0:256], hsrc[:, k, 128 * ta:128 * ta + 128],
                             w2[:, k, :], start=(k == 0), stop=(k == 3))
        res = xownN[:, ta, :] if xownN is not None else x1N_src[:, ta, :]
        src = pm[:, 0:256]
        if not fast and b128_name is not None:
            tb = spool.tile([P, 256], F32, tag="tb", name="tb", bufs=2)
            nc.vector.tensor_tensor(tb[:], src, cin[b128_name], TT.add)
            src = tb[:]
        xr = spool.tile([P, 256], F32, tag=f"xr{tt}", name="xr")
        nc.vector.scalar_tensor_tensor(xr[:], src, 0.0, res, TT.add, TT.add,
                                       accum_out=mu_raw[:, tt:tt + 1])
        sq = spool.tile([P, 256], F32, tag="sq", name="sq", bufs=2)
        nc.vector.scalar_tensor_tensor(sq[:], xr[:], 0.0, xr[:], TT.add, TT.mult,
                                       accum_out=s2_raw[:, tt:tt + 1])
        xrs.append(xr)
    # fused stats: var = s2/256 - (mu_raw/256)^2; istd = exp(-ln(var+eps)/2);
    # nmui = -mu*istd  (5 serial ops instead of 9 — this chain is the
    # latency-critical part of the exposed tail)
    var = spool.tile([P, 2], F32, tag="var", bufs=2)
    istd = spool.tile([P, 2], F32, tag="istd", bufs=2)
    nmui = spool.tile([P, 2], F32, tag="nmui", bufs=2)
    nc.vector.scalar_tensor_tensor(var[:], mu_raw[:], 1.0 / 65536.0, mu_raw[:],
                                   TT.mult, TT.mult)
    nc.vector.scalar_tensor_tensor(var[:], s2_raw[:], 1.0 / 256.0, var[:],
                                   TT.mult, TT.subtract)
    nc.scalar.activation(var[:], var[:], AF.Ln, bias=cin["eps1"])
    nc.scalar.activation(istd[:], var[:], AF.Exp, scale=-0.5)
    nc.vector.scalar_tensor_tensor(nmui[:], mu_raw[:], -1.0 / 256.0, istd[:],
                                   TT.mult, TT.mult)
    for tt in range(2):
        ta = 4 * qt + 2 * uu + tt
        xr = xrs[tt]
        if fast:
            nc.scalar.activation(dest[:, ta, :], xr[:], AF.Identity,
                                 bias=nmui[:, tt:tt + 1], scale=istd[:, tt:tt + 1])
        else:
            nc.scalar.activation(xr[:], xr[:], AF.Identity,
                                 bias=nmui[:, tt:tt + 1], scale=istd[:, tt:tt + 1])
            nc.vector.tensor_tensor(xr[:], xr[:], cin[ln_prefix + "g128"], TT.mult)
            nc.vector.tensor_tensor(dest[:, ta, :], xr[:], cin[ln_prefix + "b128"],
                                    TT.add)


# ======================================================================
# Host side
# ======================================================================

_NC = {}


def _get_nc(fast):
    if fast not in _NC:
        _NC[fast] = build(fast)
    return _NC[fast]


def _img_T(mat):
    """[R, C] fp32 (R = k*128) -> SBUF image [128, k*C] for T-layout tiles."""
    R, C = mat.shape
    k = R // 128
    return np.ascontiguousarray(
        mat.reshape(k, 128, C).transpose(1, 0, 2).reshape(128, k * C))


def _img_N(mat):
    """[T, F] (T = t*128) -> SBUF image [128, t*F] for N-layout tiles."""
    T, F = mat.shape
    t = T // 128
    return np.ascontiguousarray(
        mat.reshape(t, 128, F).transpose(1, 0, 2).reshape(128, t * F))


def _bias_cols(b):
    """[k*128] -> [128, k] per-partition column layout."""
    return np.ascontiguousarray(b.reshape(-1, 128).T)


def _in_maps(x, g_in_w, g_in_b, g_out_w, g_out_b,
             t_in_w, t_in_b, t_out_w, t_out_b,
             fus_w1, fus_b1, fus_w2, fus_b2,
             ffn_w1, ffn_b1, ffn_w2, ffn_b2,
             gn_g, gn_b, fn_g, fn_b):
    x = np.asarray(x, np.float32)
    f32 = lambda a: np.asarray(a, np.float32)
    bf = lambda a: np.asarray(a, np.float32).astype(BF_NP)

    # fast path: every remaining device-side bias is zero and LN affine
    # params are identity (guaranteed by the problem's input fills; the
    # generic path handles anything else)
    fast = bool(
        np.all(f32(g_in_b)[512:768] == 0) and np.all(f32(t_in_b)[512:768] == 0)
        and np.all(f32(g_out_b) == 0) and np.all(f32(t_out_b) == 0)
        and np.all(f32(ffn_b2) == 0)
        and np.all(f32(gn_g) == 1) and np.all(f32(gn_b) == 0)
        and np.all(f32(fn_g) == 1) and np.all(f32(fn_b) == 0))

    # shared (same on all cores) tensors
    shared = {
        "wgq": bf(_img_T(f32(g_in_w)[0:256].T)),
        "wgk": bf(_img_T(f32(g_in_w)[256:512].T)),
        "wgv": bf(_img_T(f32(g_in_w)[512:768].T)),
        "wtqk": bf(_img_T(f32(t_in_w)[0:512].T)),
        "wtv": bf(_img_T(f32(t_in_w)[512:768].T)),
        "wgo": bf(_img_T(f32(g_out_w).T)),
        "wto": bf(_img_T(f32(t_out_w).T)),
        "wf1": bf(_img_T(f32(fus_w1).T)),
        "wf2": bf(_img_T(f32(fus_w2).T)),
        "wn1": bf(_img_T(f32(ffn_w1).T)),
        "wn2": bf(_img_T(f32(ffn_w2).T)),
    }
    packs = {
        "bgq": _bias_cols(f32(g_in_b)[0:256]),
        "bgk": _bias_cols(f32(g_in_b)[256:512]),
        "btqk": _bias_cols(f32(t_in_b)[0:512]),
        "bgo": _bias_cols(f32(g_out_b)),
        "bto": _bias_cols(f32(t_out_b)),
        "bf1": _bias_cols(f32(fus_b1)),
        "nbf1": _bias_cols(-f32(fus_b1)),
        "bn1": _bias_cols(f32(ffn_b1)),
        "nbn1": _bias_cols(-f32(ffn_b1)),
        "bgv128": np.broadcast_to(f32(g_in_b)[512:768], (P, 256)),
        "btv128": np.broadcast_to(f32(t_in_b)[512:768], (P, 256)),
        "bn2128": np.broadcast_to(f32(ffn_b2), (P, 256)),
        "gng128": np.broadcast_to(f32(gn_g), (P, 256)),
        "gnb128": np.broadcast_to(f32(gn_b), (P, 256)),
        "fng128": np.broadcast_to(f32(fn_g), (P, 256)),
        "fnb128": np.broadcast_to(f32(fn_b), (P, 256)),
    }
    shared["packF"] = np.ascontiguousarray(
        np.concatenate([packs[n] for n, _ in PACKF], axis=1).astype(np.float32))
    # band mask: key row j valid for query qq iff qq <= j <= qq+4
    jj = np.arange(P)[:, None]
    qq = np.arange(LB)[None, :]
    bandA = ((qq <= jj) & (jj <= qq + 4)).astype(np.float32)
    bandF = bandA.copy()
    bandF[0:2] = 0.0           # keys at tokens -2, -1 (first block, first half)
    bandL = bandA.copy()
    bandL[34:36] = 0.0         # block-8 keys x_q rows 1026, 1027 (= S, S+1)

    in_maps = []
    for c in range(8):
        b, hh = c // 2, c % 2
        t0 = 1024 * hh
        xb = x[b]                                    # [2048, 256]
        xq = np.zeros((XQ + 4, D), np.float32)       # rows = x_q tokens t0-2 ..
        lo, hi = max(0, t0 - 2), min(S, t0 + XQ + 2)
        xq[lo - (t0 - 2):hi - (t0 - 2)] = xb[lo:hi]
        xq = xq[:XQ]                                 # guard: only XQ rows used
        m = dict(shared)
        m["xkvT"] = bf(_img_T(xb.T))
        m["xqT"] = bf(_img_T(xq.T))
        m["xownN"] = bf(_img_N(xb[t0:t0 + 1024] + f32(fus_b2)[None, :]))
        m["packB"] = np.ascontiguousarray(np.concatenate(
            [bandA, bandF if hh == 0 else bandA, bandL if hh == 1 else bandA],
            axis=1)).astype(BF_NP)
        in_maps.append(m)
    return in_maps, fast


def _assemble(results):
    out = np.zeros((B, S, D), np.float32)
    for c in range(8):
        b, hh = c // 2, c % 2
        img = results[c]["out"]                      # [128, 2048]
        chunk = img.reshape(P, 8, 256).transpose(1, 0, 2).reshape(1024, 256)
        out[b, 1024 * hh:1024 * hh + 1024] = chunk
    return out


_LAST_RES = None


def kernel(**inputs):
    global _LAST_RES
    in_maps, fast = _in_maps(**inputs)
    nc = _get_nc(fast)
    res = run_bass_kernel_spmd(nc, in_maps, core_ids=list(range(8)))
    _LAST_RES = res
    return _assemble(res.results)


# revision 47
# speedup vs baseline: 1.0886x; 1.0181x over previous
"""Trainium2 Bass kernel for nn_EnhancedTransformerBlock_51917564674691.

Reference block (B=4, S=2048, D=256):
  x_global = global_mha(x, 8 heads, hd=32)          # dense S x S attention
  x_local  = local_mha(x, 4 heads, hd=64, window=5) # banded attention
  x_fused  = MLP_512(silu) over concat([x_global, x_local])
  x        = LN(x + x_fused); x = LN(x + FFN(x)); return x

Sharding: 8 cores = 4 batches x 2 sequence-halves. Each core computes the
full-batch K/V for global attention (needs all 2048 keys) and produces the
output for its 1024 tokens.

Layouts: "T-layout" = [feature partitions, token free] throughout the matmul
chain; host pre-transposes x and all weights into SBUF-image form so every
DMA is a contiguous [128, F] block. Attention internals are bf16 (fp32 PSUM
accumulation everywhere).

Pipelining: the global-attention score PSUM is split into two [128,1024]
half-tiles (2 banks each, double-buffered) so the PE score matmuls of key
tile kt+1 overlap the softmax exp of key tile kt on the scalar engine; the
AV accumulation for the second half lags one iteration so the PE never
waits on an in-flight exp. Softmax denominators come from ones-columns in
the V weights; divisions use the fast DVE reciprocal approximation.
Elementwise work is spread across Vector, GpSimd(Pool) and Scalar engines.
"""
import os
import numpy as np
import ml_dtypes

import concourse.bass as bass
import concourse.tile as tile
from concourse import bacc, mybir
from concourse.bass_utils import run_bass_kernel_spmd
from concourse.masks import make_identity

P = 128
BF = mybir.dt.bfloat16
F32 = mybir.dt.float32
FP8 = mybir.dt.float8e4
I8 = mybir.dt.int8
BF_NP = ml_dtypes.bfloat16

B, S, D = 4, 2048, 256
TQ = 1024           # tokens per core
XQ = 1152           # padded x_q length (own tokens + halo, zero padded)
NQT = 2             # global q tiles of 512
NKT = 16            # global key tiles of 128
GSC = 1.0 / np.sqrt(32.0)   # global attention scale
LSC = 0.125                 # local attention scale (1/sqrt(64))
# Schraudolph fast-exp constants targeting a bf16 bit pattern:
# bf16bits(exp(GSC*x)) ~= round(x * 128*GSC*log2(e) + (127*128 - 5.5))
A16 = float(128.0 * GSC * np.log2(np.e))
B16 = 16250.5
LB = 124            # local block queries
NLB = 9             # local blocks (9*124 = 1116 >= 1024)
EPS = 1e-5

AF = mybir.ActivationFunctionType
TT = mybir.AluOpType

# small per-partition tensors packed into one f32 image (name -> width)
PACKF = [
    ("bgq", 2), ("bgk", 2), ("btqk", 4), ("bgo", 2), ("bto", 2),
    ("bf1", 4), ("nbf1", 4), ("bn1", 4), ("nbn1", 4),
    ("bgv128", 256), ("btv128", 256), ("bn2128", 256),
    ("gng128", 256), ("gnb128", 256), ("fng128", 256), ("fnb128", 256),
]
PACKF_W = sum(w for _, w in PACKF)
PACKB = [("bandA", LB), ("bandF", LB), ("bandL", LB)]
PACKB_W = sum(w for _, w in PACKB)

# name -> (shape, np dtype) of per-core DRAM inputs (all SBUF-image [128, F]),
# in DMA issue order (alternating between the two HW DGE queues)
INPUT_SPECS = {
    "xqT": ((P, 2 * XQ), BF_NP),      # x_q.T padded      (own + halo, T-layout)
    "wgq": ((P, 2 * 256), BF_NP),
    "wgk": ((P, 2 * 256), BF_NP),
    "xkvT": ((P, 2 * 2048), BF_NP),   # x[b].T            (full batch, T-layout)
    "wtqk": ((P, 2 * 512), BF_NP),
    "wgv": ((P, 2 * 256), BF_NP),
    "wtv": ((P, 2 * 256), BF_NP),
    "packB": ((P, PACKB_W), BF_NP),
    "packF": ((P, PACKF_W), np.float32),
    "wgo": ((P, 2 * 256), BF_NP),
    "wto": ((P, 2 * 256), BF_NP),
    "wf1": ((P, 4 * 512), BF_NP),
    "wf2": ((P, 4 * 256), BF_NP),
    "wn1": ((P, 2 * 512), BF_NP),
    "wn2": ((P, 4 * 256), BF_NP),
    "xownN": ((P, 8 * 256), BF_NP),   # x own tokens + fus_b2 (N-layout)
}


def _patch_act_tables():
    """Make Exp and Ln resolve to the combined natural_log_exp_and_others set
    so the table-load pass emits ONE load instead of thrashing between
    exp_and_others and natural_log (9 loads, ~2.7us each + ACT drains)."""
    import concourse.hw_specs as hs
    if getattr(hs, "_act_tables_patched", False):
        return
    orig = hs.get_activation_tables

    def patched(module_arch):
        t = dict(orig(module_arch))
        exp = mybir.ActivationFunctionType.Exp
        ln = mybir.ActivationFunctionType.Ln
        for name in ("exp_and_others", "exp_and_friends"):
            if name in t:
                t[name] = t[name] - {exp}
        if "natural_log" in t:
            t["natural_log"] = t["natural_log"] - {ln}
        return t

    hs.get_activation_tables = patched
    import concourse.bacc as bc
    bc.get_activation_tables = patched
    hs._act_tables_patched = True


def build(fast):
    _patch_act_tables()
    nc = bacc.Bacc("TRN2", target_bir_lowering=False, debug=False, num_devices=8)
    dram = {}
    for name, (shape, npdt) in INPUT_SPECS.items():
        dram[name] = nc.dram_tensor(
            name, list(shape), mybir.dt.from_np(np.dtype(npdt)), kind="ExternalInput"
        ).ap()
    out_dram = nc.dram_tensor("out", [P, 8 * 256], F32, kind="ExternalOutput").ap()

    with tile.TileContext(nc) as tc:
        _emit(nc, tc, dram, out_dram, fast)
    nc.compile()
    return nc


def _emit(nc, tc, dram, out_dram, fast):
    from contextlib import ExitStack
    ctx = ExitStack()

    cpool = ctx.enter_context(tc.tile_pool(name="const", bufs=1))
    wpool = ctx.enter_context(tc.tile_pool(name="work", bufs=1))
    spool = ctx.enter_context(tc.tile_pool(name="scratch", bufs=4))
    epool = ctx.enter_context(tc.tile_pool(name="exps", bufs=2))
    pp = ctx.enter_context(tc.tile_pool(name="ps", bufs=1, space="PSUM"))

    def _kernel_body():
            # ---- load constants / inputs --------------------------------------
            # split DMAs across the two HW DGE queues (SP + ACT) in dependency
            # order; small tensors arrive pre-packed as two images
            cin = {}
            for n, (name, (shape, npdt)) in enumerate(INPUT_SPECS.items()):
                t = cpool.tile(list(shape), mybir.dt.from_np(np.dtype(npdt)), tag=name)
                eng = nc.sync if n % 2 == 0 else nc.scalar
                eng.dma_start(t[:], dram[name])
                cin[name] = t[:]
            off = 0
            for name, wd in PACKF:
                cin[name] = cin["packF"][:, off:off + wd]
                off += wd
            off = 0
            for name, wd in PACKB:
                cin[name] = cin["packB"][:, off:off + wd]
                off += wd

            ones_bf = cpool.tile([P, 64], BF, tag="ones_bf")
            nc.vector.memset(ones_bf[:], 1.0)
            eps_col = cpool.tile([P, 1], F32, tag="eps_col")
            nc.vector.memset(eps_col[:], EPS)
            cin["eps1"] = eps_col[:]
            ident = cpool.tile([P, P], F32, tag="ident")
            make_identity(nc, ident[:])

            # reshaped views of inputs
            xkvT = cin["xkvT"].rearrange("p (k n) -> p k n", k=2)     # [128,2,2048] bf
            xqT = cin["xqT"].rearrange("p (k n) -> p k n", k=2)       # [128,2,1152]
            xownN = cin["xownN"].rearrange("p (t f) -> p t f", t=8)   # [128,8,256] bf
            w = {k: cin[k].rearrange("p (k2 n) -> p k2 n", k2=2)
                 for k in ("wgq", "wgk", "wgv", "wtqk", "wtv", "wgo", "wto", "wn1")}
            w["wf1"] = cin["wf1"].rearrange("p (k2 n) -> p k2 n", k2=4)
            w["wf2"] = cin["wf2"].rearrange("p (k2 n) -> p k2 n", k2=4)
            w["wn2"] = cin["wn2"].rearrange("p (k2 n) -> p k2 n", k2=4)

            # ---- persistent intermediates ------------------------------------
            qT = wpool.tile([P, 2, 1024], BF, tag="qT")
            kT = wpool.tile([P, 2, 2048], BF, tag="kT")
            v_aug = wpool.tile([P, NKT, 8, 64], BF, tag="v_aug")
            qkL = wpool.tile([P, 4, XQ], BF, tag="qkL")
            vL = wpool.tile([P, NLB, 256], BF, tag="vL")
            g_oT = wpool.tile([P, 2, 1024], BF, tag="g_oT")
            l_oT = wpool.tile([P, 2, 1024], BF, tag="l_oT")
            combT = wpool.tile([P, 4, 1024], BF, tag="combT")
            h1s = wpool.tile([P, 4, 1024], BF, tag="h1s")
            x1N = wpool.tile([P, 8, 256], F32, tag="x1N")
            x1T = wpool.tile([P, 2, 1024], BF, tag="x1T")
            h2s = wpool.tile([P, 4, 1024], BF, tag="h2s")
            out_sb = wpool.tile([P, 8, 256], F32, tag="out_sb")

            def ps_sc():
                return pp.tile([P, 1024], F32, tag="sc", bufs=2, name="ps_sc")

            def ps_av():
                return pp.tile([P, 512], F32, tag="av", bufs=2, name="ps_av")

            def ps_sm():
                return pp.tile([P, 512], F32, tag="sm", bufs=2, name="ps_sm")

            def bias_bc(name, m, n):
                return cin[name][:, m:m + 1].to_broadcast([P, n])

            # ---- qkv projections (global) ------------------------------------
            # qT over own tokens first (x_q rows 2..1026)
            for m in range(2):
                for nt in range(2):
                    pm = ps_sm()
                    for k in range(2):
                        nc.tensor.matmul(pm[:], w["wgq"][:, k, 128 * m:128 * m + 128],
                                         xqT[:, k, 2 + 512 * nt:2 + 512 * nt + 512],
                                         start=(k == 0), stop=(k == 1))
                    if fast:
                        nc.vector.tensor_copy(qT[:, m, 512 * nt:512 * nt + 512], pm[:])
                    else:
                        nc.scalar.activation(qT[:, m, 512 * nt:512 * nt + 512], pm[:],
                                             AF.Identity, bias=cin["bgq"][:, m:m + 1])
            # kT = Wk @ x^T over full batch
            for m in range(2):
                for nt in range(4):
                    pm = ps_sm()
                    for k in range(2):
                        nc.tensor.matmul(pm[:], w["wgk"][:, k, 128 * m:128 * m + 128],
                                         xkvT[:, k, 512 * nt:512 * nt + 512],
                                         start=(k == 0), stop=(k == 1))
                    if fast:
                        nc.vector.tensor_copy(kT[:, m, 512 * nt:512 * nt + 512], pm[:])
                    else:
                        nc.scalar.activation(kT[:, m, 512 * nt:512 * nt + 512], pm[:],
                                             AF.Identity, bias=cin["bgk"][:, m:m + 1])
            # ---- qkv projections (local) -------------------------------------
            for m in range(4):
                for nt in range(3):
                    pm = ps_sm()
                    for k in range(2):
                        nc.tensor.matmul(pm[:, 0:384], w["wtqk"][:, k, 128 * m:128 * m + 128],
                                         xqT[:, k, 384 * nt:384 * nt + 384],
                                         start=(k == 0), stop=(k == 1))
                    nc.scalar.activation(qkL[:, m, 384 * nt:384 * nt + 384], pm[:, 0:384],
                                         AF.Identity, bias=cin["btqk"][:, m:m + 1])
            # v (N-layout, augmented with ones columns): v[key, f] over full batch
            nc.vector.memset(v_aug[:, :, :, 32:64], 1.0)
            for mt in range(16):
                pm = ps_sm()
                for k in range(2):
                    nc.tensor.matmul(pm[:, 0:256], xkvT[:, k, 128 * mt:128 * mt + 128],
                                     w["wgv"][:, k, :], start=(k == 0), stop=(k == 1))
                dst = v_aug[:, mt, :, 0:32]
                src = pm[:, 0:256].rearrange("p (h d) -> p h d", h=8)
                if fast:
                    nc.vector.tensor_copy(dst, src)
                else:
                    nc.vector.tensor_tensor(
                        dst, src,
                        cin["bgv128"].rearrange("p (h d) -> p h d", h=8), TT.add)
            for blk in range(NLB):
                pm = ps_sm()
                for k in range(2):
                    nc.tensor.matmul(pm[:, 0:256], xqT[:, k, 124 * blk:124 * blk + 128],
                                     w["wtv"][:, k, :], start=(k == 0), stop=(k == 1))
                if fast:
                    nc.vector.tensor_copy(vL[:, blk, :], pm[:, 0:256])
                else:
                    nc.vector.tensor_tensor(vL[:, blk, :], pm[:, 0:256],
                                            cin["btv128"], TT.add)

            # ---- local attention ---------------------------------------------
            # (pipelined: score PSUM halves double-buffered, exp per half,
            #  band mask on Pool, fast reciprocal on DVE)
            for blk in range(NLB):
                k0 = 124 * blk
                q0 = 2 + 124 * blk
                qn = 32 if blk == NLB - 1 else LB  # valid queries in this block
                eloc = epool.tile([P, 4, LB], BF, tag="eloc", bufs=2)
                for half in range(2):
                    psc = ps_sc()
                    pv = psc[:].rearrange("p (k n) -> p k n", k=2)
                    for r in range(2):
                        l = 2 * half + r
                        nc.tensor.matmul(pv[:, r, 0:LB],
                                         qkL[64 * r:64 * r + 64, 2 + half, k0:k0 + 128],
                                         qkL[64 * r:64 * r + 64, half, q0:q0 + LB],
                                         start=True, stop=True, tile_position=(64 * r, 0))
                    nc.scalar.activation(eloc[:, 2 * half:2 * half + 2, :],
                                         pv[:, :, 0:LB], AF.Exp, scale=LSC)
                band = ("bandF" if blk == 0 else
                        ("bandL" if blk == NLB - 1 else "bandA"))
                nc.vector.tensor_tensor(eloc[:], eloc[:],
                                        cin[band][:, None, :].to_broadcast([P, 4, LB]),
                                        TT.mult)  # cin[band] is an AP slice of packB
                pav = [ps_av(), ps_av()]
                pde = [ps_sm(), ps_sm()]
                for l in range(4):
                    pr, c = l // 2, l % 2
                    nc.tensor.matmul(pav[pr][64 * c:64 * c + 64, 0:LB],
                                     vL[:, blk, 64 * l:64 * l + 64], eloc[:, l, :],
                                     start=True, stop=True, tile_position=(0, 64 * c))
                    nc.tensor.matmul(pde[pr][64 * c:64 * c + 64, 0:LB],
                                     ones_bf[:], eloc[:, l, :],
                                     start=True, stop=True, tile_position=(0, 64 * c))
                for pr in range(2):
                    rec = spool.tile([P, LB], F32, tag="lrec", bufs=2)
                    nc.vector.reciprocal_approx_fast(rec[:], pde[pr][:, 0:LB])
                    nc.vector.tensor_tensor(l_oT[:, pr, k0:k0 + qn], pav[pr][:, 0:qn],
                                            rec[:, 0:qn], TT.mult)

            # ---- per-chunk: global attention + MLP tail ----------------------
            for qt in range(NQT):
                qsl = slice(512 * qt, 512 * qt + 512)
                for hg in range(2):
                    pav = [ps_av(), ps_av()]
                    lag = None  # (eg tile, kt) for the lagged half-B AV matmuls

                    def av_half(p2, eg, kt):
                        for c in range(2):
                            h = 4 * hg + 2 * p2 + c
                            # skip_group_check: CoreSim's zero-region tracker is
                            # partition-blind (any two concurrent groups per bank
                            # conflict); HW has per-element has_written bits and
                            # the 64-offset dual-group pattern is exact on HW.
                            nc.tensor.matmul(pav[p2][64 * c:64 * c + 64, :],
                                             v_aug[:, kt, h, :],
                                             eg[:, 512 * c:512 * c + 512],
                                             start=(kt == 0), stop=(kt == NKT - 1),
                                             tile_position=(0, 64 * c),
                                             skip_group_check=True)

                    for kt in range(NKT):
                        pscA = ps_sc()
                        for r in range(2):
                            nc.tensor.matmul(
                                pscA[:, 512 * r:512 * r + 512],
                                kT[32 * r:32 * r + 32, hg, 128 * kt:128 * kt + 128],
                                qT[32 * r:32 * r + 32, hg, qsl],
                                start=True, stop=True, tile_position=(32 * r, 0))
                        egA = epool.tile([P, 1024], BF, tag="egA", bufs=2)
                        nc.scalar.activation(egA[:], pscA[:], AF.Exp, scale=GSC)
                        pscB = ps_sc()
                        for r in range(2):
                            hc = 2 + r
                            nc.tensor.matmul(
                                pscB[:, 512 * r:512 * r + 512],
                                kT[32 * hc:32 * hc + 32, hg, 128 * kt:128 * kt + 128],
                                qT[32 * hc:32 * hc + 32, hg, qsl],
                                start=True, stop=True, tile_position=(32 * hc, 0))
                        # Schraudolph fast exp on DVE: scale+shift the score so
                        # the rounded int16 IS the bf16 bit pattern of
                        # exp(GSC*score) (softmax ratio cancels the
                        # piecewise-linear mantissa error)
                        egB = epool.tile([P, 1024], mybir.dt.int16, tag="egB",
                                         bufs=2)
                        nc.vector.tensor_scalar(egB[:], pscB[:], A16, B16,
                                                TT.mult, TT.add)
                        # AV lags one kt behind so the PE never waits on an
                        # in-flight exp
                        if lag is not None:
                            av_half(0, lag[0][:], lag[2])
                            av_half(1, lag[1][:].bitcast(BF), lag[2])
                        lag = (egA, egB, kt)
                    av_half(0, lag[0][:], lag[2])
                    av_half(1, lag[1][:].bitcast(BF), lag[2])

                    # normalize: one fast reciprocal per accumulator (denominator
                    # rows 32:64 / 96:128; extra rows are unused garbage)
                    for p2 in range(2):
                        rec = spool.tile([P, 512], F32, tag="grec", bufs=2)
                        nc.vector.reciprocal_approx_fast(rec[:], pav[p2][:])
                        nc.vector.tensor_tensor(g_oT[64 * p2:64 * p2 + 32, hg, qsl],
                                                pav[p2][0:32, :], rec[32:64, :], TT.mult)
                        nc.vector.tensor_tensor(g_oT[64 * p2 + 32:64 * p2 + 64, hg, qsl],
                                                pav[p2][64:96, :], rec[96:128, :], TT.mult)

                # ---- out projections -> combT --------------------------------
                for m in range(2):
                    pm = ps_sm()
                    for k in range(2):
                        nc.tensor.matmul(pm[:], w["wgo"][:, k, 128 * m:128 * m + 128],
                                         g_oT[:, k, qsl],
                                         start=(k == 0), stop=(k == 1))
                    if fast:
                        nc.scalar.activation(combT[:, m, qsl], pm[:], AF.Identity)
                    else:
                        nc.vector.tensor_tensor(combT[:, m, qsl], pm[:],
                                                bias_bc("bgo", m, 512), TT.add)
                for m in range(2):
                    pm = ps_sm()
                    for k in range(2):
                        nc.tensor.matmul(pm[:], w["wto"][:, k, 128 * m:128 * m + 128],
                                         l_oT[:, k, qsl],
                                         start=(k == 0), stop=(k == 1))
                    if fast:
                        nc.scalar.activation(combT[:, 2 + m, qsl], pm[:], AF.Identity)
                    else:
                        nc.vector.tensor_tensor(combT[:, 2 + m, qsl], pm[:],
                                                bias_bc("bto", m, 512), TT.add)

                # ---- fused MLP gemm1 + silu ----------------------------------
                for m in range(4):
                    pm = ps_sm()
                    for k in range(4):
                        nc.tensor.matmul(pm[:], w["wf1"][:, k, 128 * m:128 * m + 128],
                                         combT[:, k, qsl],
                                         start=(k == 0), stop=(k == 3))
                    _silu(nc, spool, h1s[:, m, qsl], pm,
                          cin["bf1"], cin["nbf1"], m, fast)

                # fused MLP gemm2 + residual + LN1, per token pair (the LN
                # stats chain of pair 0 overlaps the gemms of pair 1), with
                # the transposes for each pair chased immediately
                for uu in range(2):
                    _g2_res_ln(nc, spool, pp, cin, qt, uu, h1s, w["wf2"], None,
                               xownN, "gn", x1N, x1N_src=None, fast=fast)
                    for tt in (2 * uu, 2 * uu + 1):
                        ta = 4 * qt + tt
                        for fh in range(2):
                            ptr = ps_sm()
                            nc.tensor.transpose(ptr[:, 0:128],
                                                x1N[:, ta, 128 * fh:128 * fh + 128],
                                                ident[:])
                            nc.scalar.activation(x1T[:, fh, 128 * ta:128 * ta + 128],
                                                 ptr[:, 0:128], AF.Identity)

                # FFN gemm1 + silu
                for m in range(4):
                    pm = ps_sm()
                    for k in range(2):
                        nc.tensor.matmul(pm[:], w["wn1"][:, k, 128 * m:128 * m + 128],
                                         x1T[:, k, qsl],
                                         start=(k == 0), stop=(k == 1))
                    _silu(nc, spool, h2s[:, m, qsl], pm,
                          cin["bn1"], cin["nbn1"], m, fast)

                # FFN gemm2 + residual(x1N) + LN2 -> out_sb, per token pair
                for uu in range(2):
                    _g2_res_ln(nc, spool, pp, cin, qt, uu, h2s, w["wn2"], "bn2128",
                               None, "fn", out_sb, x1N_src=x1N, fast=fast)
                    nc.sync.dma_start(
                        out_dram[:, 1024 * qt + 512 * uu:1024 * qt + 512 * uu + 512],
                        out_sb[:, 4 * qt + 2 * uu:4 * qt + 2 * uu + 2, :]
                        .rearrange("p t f -> p (t f)"))

    REPEAT = int(os.environ.get("KREPEAT", "1"))
    if REPEAT > 1:
        with tc.For_i(0, REPEAT, 1):
            _kernel_body()
    else:
        _kernel_body()
    ctx.close()


def _silu(nc, spool, out_ap, pm, b_t, nb_t, m, fast):
    """out = silu(pm + b) where b is per-partition bias column m.

    silu(y) = y / (1 + exp(-y)); exp on ACT, then +1, fast reciprocal and
    the final (pm + b) * r on DVE."""
    src = pm[:]
    e = spool.tile([P, 512], F32, tag="se", name="se", bufs=2)
    bias = 0.0 if fast else nb_t[:, m:m + 1]
    nc.scalar.activation(e[:], src, AF.Exp, bias=bias, scale=-1.0)
    nc.vector.tensor_scalar_add(e[:], e[:], 1.0)
    r = spool.tile([P, 512], F32, tag="sr", name="sr", bufs=2)
    nc.vector.reciprocal_approx_fast(r[:], e[:])
    if fast:
        nc.vector.tensor_tensor(out_ap, src, r[:], TT.mult)
    else:
        nc.vector.scalar_tensor_tensor(out_ap, src, b_t[:, m:m + 1], r[:],
                                       TT.add, TT.mult)


def _g2_res_ln(nc, spool, pp, cin, qt, uu, hsrc, w2, b128_name, xownN, ln_prefix,
               dest, x1N_src, fast):
    """gemm2 (contract 512 -> 256, N-layout out) + bias + residual + layernorm
    for the token pair (2*uu, 2*uu+1) of chunk qt.

    residual = xownN[:, ta, :] (already includes fus_b2, host-folded) or
    x1N_src[:, ta, :] (+ b128 on device unless fast).
    dest[:, ta, :] = LN(res + gemm2_out [+ b128]) [* g128 + b128_ln]
    The (x - mu) * istd normalize runs as ONE scalar-engine activation with
    per-partition scale/bias; sums for mu/var come fused out of the two DVE
    scalar_tensor_tensor ops (accum_out)."""
    xrs = []
    mu_raw = spool.tile([P, 2], F32, tag="mu_raw", bufs=2)
    s2_raw = spool.tile([P, 2], F32, tag="s2_raw", bufs=2)
    for tt in range(2):
        ta = 4 * qt + 2 * uu + tt
        pm = pp.tile([P, 512], F32, tag="sm", bufs=2, name="ps_sm")
        for k in range(4):
            nc.tensor.matmul(pm[:, 0:256], hsrc[:, k, 128 * ta:128 * ta + 128],
                             w2[:, k, :], start=(k == 0), stop=(k == 3))
        res = xownN[:, ta, :] if xownN is not None else x1N_src[:, ta, :]
        src = pm[:, 0:256]
        if not fast and b128_name is not None:
            tb = spool.tile([P, 256], F32, tag="tb", name="tb", bufs=2)
            nc.vector.tensor_tensor(tb[:], src, cin[b128_name], TT.add)
            src = tb[:]
        xr = spool.tile([P, 256], F32, tag=f"xr{tt}", name="xr")
        nc.vector.scalar_tensor_tensor(xr[:], src, 0.0, res, TT.add, TT.add,
                                       accum_out=mu_raw[:, tt:tt + 1])
        sq = spool.tile([P, 256], F32, tag="sq", name="sq", bufs=2)
        nc.vector.scalar_tensor_tensor(sq[:], xr[:], 0.0, xr[:], TT.add, TT.mult,
                                       accum_out=s2_raw[:, tt:tt + 1])
        xrs.append(xr)
    # fused stats: var = s2/256 - (mu_raw/256)^2; istd = exp(-ln(var+eps)/2);
    # nmui = -mu*istd  (5 serial ops instead of 9 — this chain is the
    # latency-critical part of the exposed tail)
    var = spool.tile([P, 2], F32, tag="var", bufs=2)
    istd = spool.tile([P, 2], F32, tag="istd", bufs=2)
    nmui = spool.tile([P, 2], F32, tag="nmui", bufs=2)
    nc.vector.scalar_tensor_tensor(var[:], mu_raw[:], 1.0 / 65536.0, mu_raw[:],
                                   TT.mult, TT.mult)
    nc.vector.scalar_tensor_tensor(var[:], s2_raw[:], 1.0 / 256.0, var[:],
                                   TT.mult, TT.subtract)
    nc.scalar.activation(var[:], var[:], AF.Ln, bias=cin["eps1"])
    nc.scalar.activation(istd[:], var[:], AF.Exp, scale=-0.5)
    nc.vector.scalar_tensor_tensor(nmui[:], mu_raw[:], -1.0 / 256.0, istd[:],
                                   TT.mult, TT.mult)
    for tt in range(2):
        ta = 4 * qt + 2 * uu + tt
        xr = xrs[tt]
        if fast:
            nc.scalar.activation(dest[:, ta, :], xr[:], AF.Identity,
                                 bias=nmui[:, tt:tt + 1], scale=istd[:, tt:tt + 1])
        else:
            nc.scalar.activation(xr[:], xr[:], AF.Identity,
                                 bias=nmui[:, tt:tt + 1], scale=istd[:, tt:tt + 1])
            nc.vector.tensor_tensor(xr[:], xr[:], cin[ln_prefix + "g128"], TT.mult)
            nc.vector.tensor_tensor(dest[:, ta, :], xr[:], cin[ln_prefix + "b128"],
                                    TT.add)


# ======================================================================
# Host side
# ======================================================================

_NC = {}


def _get_nc(fast):
    if fast not in _NC:
        _NC[fast] = build(fast)
    return _NC[fast]


def _img_T(mat):
    """[R, C] fp32 (R = k*128) -> SBUF image [128, k*C] for T-layout tiles."""
    R, C = mat.shape
    k = R // 128
    return np.ascontiguousarray(
        mat.reshape(k, 128, C).transpose(1, 0, 2).reshape(128, k * C))


def _img_N(mat):
    """[T, F] (T = t*128) -> SBUF image [128, t*F] for N-layout tiles."""
    T, F = mat.shape
    t = T // 128
    return np.ascontiguousarray(
        mat.reshape(t, 128, F).transpose(1, 0, 2).reshape(128, t * F))


def _bias_cols(b):
    """[k*128] -> [128, k] per-partition column layout."""
    return np.ascontiguousarray(b.reshape(-1, 128).T)


def _in_maps(x, g_in_w, g_in_b, g_out_w, g_out_b,
             t_in_w, t_in_b, t_out_w, t_out_b,
             fus_w1, fus_b1, fus_w2, fus_b2,
             ffn_w1, ffn_b1, ffn_w2, ffn_b2,
             gn_g, gn_b, fn_g, fn_b):
    x = np.asarray(x, np.float32)
    f32 = lambda a: np.asarray(a, np.float32)
    bf = lambda a: np.asarray(a, np.float32).astype(BF_NP)

    # fast path: every remaining device-side bias is zero and LN affine
    # params are identity (guaranteed by the problem's input fills; the
    # generic path handles anything else)
    fast = bool(
        np.all(f32(g_in_b)[512:768] == 0) and np.all(f32(t_in_b)[512:768] == 0)
        and np.all(f32(g_out_b) == 0) and np.all(f32(t_out_b) == 0)
        and np.all(f32(ffn_b2) == 0)
        and np.all(f32(gn_g) == 1) and np.all(f32(gn_b) == 0)
        and np.all(f32(fn_g) == 1) and np.all(f32(fn_b) == 0))

    # shared (same on all cores) tensors
    shared = {
        "wgq": bf(_img_T(f32(g_in_w)[0:256].T)),
        "wgk": bf(_img_T(f32(g_in_w)[256:512].T)),
        "wgv": bf(_img_T(f32(g_in_w)[512:768].T)),
        "wtqk": bf(_img_T(f32(t_in_w)[0:512].T)),
        "wtv": bf(_img_T(f32(t_in_w)[512:768].T)),
        "wgo": bf(_img_T(f32(g_out_w).T)),
        "wto": bf(_img_T(f32(t_out_w).T)),
        "wf1": bf(_img_T(f32(fus_w1).T)),
        "wf2": bf(_img_T(f32(fus_w2).T)),
        "wn1": bf(_img_T(f32(ffn_w1).T)),
        "wn2": bf(_img_T(f32(ffn_w2).T)),
    }
    packs = {
        "bgq": _bias_cols(f32(g_in_b)[0:256]),
        "bgk": _bias_cols(f32(g_in_b)[256:512]),
        "btqk": _bias_cols(f32(t_in_b)[0:512]),
        "bgo": _bias_cols(f32(g_out_b)),
        "bto": _bias_cols(f32(t_out_b)),
        "bf1": _bias_cols(f32(fus_b1)),
        "nbf1": _bias_cols(-f32(fus_b1)),
        "bn1": _bias_cols(f32(ffn_b1)),
        "nbn1": _bias_cols(-f32(ffn_b1)),
        "bgv128": np.broadcast_to(f32(g_in_b)[512:768], (P, 256)),
        "btv128": np.broadcast_to(f32(t_in_b)[512:768], (P, 256)),
        "bn2128": np.broadcast_to(f32(ffn_b2), (P, 256)),
        "gng128": np.broadcast_to(f32(gn_g), (P, 256)),
        "gnb128": np.broadcast_to(f32(gn_b), (P, 256)),
        "fng128": np.broadcast_to(f32(fn_g), (P, 256)),
        "fnb128": np.broadcast_to(f32(fn_b), (P, 256)),
    }
    shared["packF"] = np.ascontiguousarray(
        np.concatenate([packs[n] for n, _ in PACKF], axis=1).astype(np.float32))
    # band mask: key row j valid for query qq iff qq <= j <= qq+4
    jj = np.arange(P)[:, None]
    qq = np.arange(LB)[None, :]
    bandA = ((qq <= jj) & (jj <= qq + 4)).astype(np.float32)
    bandF = bandA.copy()
    bandF[0:2] = 0.0           # keys at tokens -2, -1 (first block, first half)
    bandL = bandA.copy()
    bandL[34:36] = 0.0         # block-8 keys x_q rows 1026, 1027 (= S, S+1)

    in_maps = []
    for c in range(8):
        b, hh = c // 2, c % 2
        t0 = 1024 * hh
        xb = x[b]                                    # [2048, 256]
        xq = np.zeros((XQ + 4, D), np.float32)       # rows = x_q tokens t0-2 ..
        lo, hi = max(0, t0 - 2), min(S, t0 + XQ + 2)
        xq[lo - (t0 - 2):hi - (t0 - 2)] = xb[lo:hi]
        xq = xq[:XQ]                                 # guard: only XQ rows used
        m = dict(shared)
        m["xkvT"] = bf(_img_T(xb.T))
        m["xqT"] = bf(_img_T(xq.T))
        m["xownN"] = bf(_img_N(xb[t0:t0 + 1024] + f32(fus_b2)[None, :]))
        m["packB"] = np.ascontiguousarray(np.concatenate(
            [bandA, bandF if hh == 0 else bandA, bandL if hh == 1 else bandA],
            axis=1)).astype(BF_NP)
        in_maps.append(m)
    return in_maps, fast


def _assemble(results):
    out = np.zeros((B, S, D), np.float32)
    for c in range(8):
        b, hh = c // 2, c % 2
        img = results[c]["out"]                      # [128, 2048]
        chunk = img.reshape(P, 8, 256).transpose(1, 0, 2).reshape(1024, 256)
        out[b, 1024 * hh:1024 * hh + 1024] = chunk
    return out


_LAST_RES = None


def kernel(**inputs):
    global _LAST_RES
    in_maps, fast = _in_maps(**inputs)
    nc = _get_nc(fast)
    res = run_bass_kernel_spmd(nc, in_maps, core_ids=list(range(8)))
    _LAST_RES = res
    return _assemble(res.results)
